# revision 1
# baseline (speedup 1.0000x reference)
"""Trainium2 Bass kernel for nn_CodaAttention (GQA attention with depth-KV
prefix, QK-norm, RoPE, XSA value-projection subtraction).

Sharding: tensor-parallel over heads across 8 cores. Core c owns q-heads
{2c, 2c+1} and kv-head c//2. Every core receives the FULL x^T (f32) and
casts the tiles it needs straight into SBUF bf16 during the projection
loads (no x AllGather, no startup collective). The kernel is emitted as a
software pipeline over 512-token chunks: projections(+RoPE/QK-norm) for
chunk n, attention for query group n, AllGather of that y chunk, then the
wo matmuls for the chunk two groups later — so PE stays busy end-to-end.
Attention uses transposed logits L^T[k, q] so softmax'd probabilities come
out directly in the lhsT layout needed by the PV matmul; QK-norm bounds
|logits| <= sqrt(128) so no max-subtraction is needed. All 1/x and
1/sqrt(x) go through reciprocal_approx_fast (DVE) or exp(-0.5*ln(x))
(Scalar) -- ln/exp/square share one activation table so the pipelined
schedule never reloads activation tables.
"""
import os
import sys

sys.path.insert(0, "/opt/trn_rl_repo")

import numpy as np

import concourse.bass as bass
import concourse.mybir as mybir
import concourse.tile as tile
from concourse import bacc

DT = mybir.dt
F32, BF16 = DT.float32, DT.bfloat16
AF = mybir.ActivationFunctionType
ALU = mybir.AluOpType

B, T, DIM = 2, 2048, 2048
H, KVH, HD = 16, 4, 128
TD = 64
NCORES = 8
HPC = H // NCORES            # q heads per core = 2
TOK = B * T                  # 4096 flattened tokens
NKD = DIM // 128             # 16 contraction tiles
NCH = T // 512               # 4 query groups (512-token chunks) per batch
SCALE = 1.0 / np.sqrt(HD)


def _build():
    nc = bacc.Bacc("TRN2", target_bir_lowering=False, debug=False,
                   num_devices=NCORES)

    # ------- I/O (host pre-transposes, pre-tiles, and pre-casts to bf16
    # so every device DMA is a large contiguous read) -------
    def inp(name, shape, dt=F32):
        return nc.dram_tensor(name, list(shape), dt,
                              kind="ExternalInput").ap()

    x_bf = inp("x_bf", (B * NCH, DIM, 512), BF16)   # x^T in 512-tok chunks
    wq_t = inp("wq_t", (HPC, NKD, 128, 128), BF16)  # pre-tiled lhsT tiles
    wk_t = inp("wk_t", (1, NKD, 128, 128), BF16)
    wv_t = inp("wv_t", (1, NKD, 128, 128), BF16)
    wo_t = inp("wo_t", (HPC, NKD, 128, 128), BF16)
    vb_ch = inp("vb_ch", (B * NCH, HD, 512))        # value_bias^T chunks, f32
    dkT_c = inp("dkT_c", (B, HD, TD), BF16)         # transposed depth_k slice
    dv_c = inp("dv_c", (B, TD, HD), BF16)
    cosT = inp("cosT", (HD, T), BF16)               # pair-duplicated cos
    sinT = inp("sinT", (HD, T), BF16)               # pair-dup sign-folded sin
    qs_c = inp("qs_c", (128, HPC))                  # q_scale per local head
    ks_c = inp("ks_c", (128, 1))                    # k_scale, bcast

    outT = nc.dram_tensor("outT", [HPC * HD, TOK], F32,
                          kind="ExternalOutput").ap()

    # ---------------- DRAM scratch ----------------
    vt_dram = nc.dram_tensor("vt_dram", [B, HD, T], BF16).ap()
    y_mine = [nc.dram_tensor(f"y_mine{i}", [HPC * HD, 1024], BF16).ap()
              for i in range(4)]
    y_all = [nc.dram_tensor(f"y_all{i}", [H * HD, 1024], BF16,
                            addr_space="Shared").ap() for i in range(4)]

    with tile.TileContext(nc) as tc:
        _emit(nc, tc, locals())
    nc.compile()
    return nc


def _emit(nc, tc, v):
    x_bf, wq_t, wk_t, wv_t, wo_t = (v["x_bf"], v["wq_t"], v["wk_t"],
                                    v["wv_t"], v["wo_t"])
    vb_ch, dkT_c, dv_c, cosT, sinT = (v["vb_ch"], v["dkT_c"], v["dv_c"],
                                      v["cosT"], v["sinT"])
    qs_c, ks_c, outT = v["qs_c"], v["ks_c"], v["outT"]
    vt_dram, y_mine, y_all = v["vt_dram"], v["y_mine"], v["y_all"]

    # ---------------- pools ----------------
    const = tc.alloc_tile_pool(name="const", bufs=1)
    wpool = tc.alloc_tile_pool(name="wpool", bufs=1)
    big = tc.alloc_tile_pool(name="big", bufs=1)
    xp = tc.alloc_tile_pool(name="xT", bufs=2)
    qp = tc.alloc_tile_pool(name="qT", bufs=2)
    rp = tc.alloc_tile_pool(name="rope", bufs=2)
    vbp = tc.alloc_tile_pool(name="vb", bufs=2)
    ap_sb = tc.alloc_tile_pool(name="attn_sb", bufs=2)
    vt_sb = tc.alloc_tile_pool(name="vt_sb", bufs=2)
    wop = tc.alloc_tile_pool(name="wo_rhs", bufs=2)
    wos = tc.alloc_tile_pool(name="wo_sb", bufs=2)
    # PSUM: 8 banks total
    pp = tc.alloc_tile_pool(name="pp", bufs=2, space="PSUM")   # proj + wo
    pl = tc.alloc_tile_pool(name="pL", bufs=2, space="PSUM")   # logit tiles
    pt = tc.alloc_tile_pool(name="pT", bufs=2, space="PSUM")   # ss/vn/dot
    py = tc.alloc_tile_pool(name="pY", bufs=1, space="PSUM")
    pz = tc.alloc_tile_pool(name="pZ", bufs=1, space="PSUM")

    # ---------------- constants ----------------
    cos_sb = const.tile([HD, T], BF16, tag="cos")
    sin_sb = const.tile([HD, T], BF16, tag="sin")
    nc.sync.dma_start(out=cos_sb[:, :], in_=cosT[:, :])
    nc.sync.dma_start(out=sin_sb[:, :], in_=sinT[:, :])
    qs_sb = const.tile([128, HPC], F32, tag="qs")
    ks_sb = const.tile([128, 1], F32, tag="ks")
    nc.sync.dma_start(out=qs_sb[:, :], in_=qs_c[:, :])
    nc.sync.dma_start(out=ks_sb[:, :], in_=ks_c[:, :])
    ones_bf = const.tile([128, 128], BF16, tag="ones")
    nc.gpsimd.memset(ones_bf[:, :], 1.0)
    eps_sb = const.tile([128, 1], F32, tag="eps")
    nc.gpsimd.memset(eps_sb[:, :], 1e-12)
    # ln(q_scale) / ln(k_scale) biases for the exp(-0.5*ln(ss)+ln(s)) rsqrt
    ln_qs = const.tile([128, HPC], F32, tag="lnqs")
    nc.scalar.activation(ln_qs[:, :], qs_sb[:, :], AF.Ln)
    ln_ks = const.tile([128, 1], F32, tag="lnks")
    nc.scalar.activation(ln_ks[:, :], ks_sb[:, :], AF.Ln)
    # 0/1 causal masks, keep where c >= p + d.
    # masks[0] (d=0): depth tile for query group 0 (j = p).
    # masks[1..5] (d=128jj-64): seq tiles straddling the causal boundary;
    # the seq grid is shifted +64 vs queries so FIVE tiles need masking.
    masks = []
    for mi, d in enumerate((0, -64, 64, 192, 320, 448)):
        m = const.tile([128, 512], BF16, tag=f"mask{mi}", name=f"mask{mi}")
        nc.gpsimd.memset(m[:, :], 1.0)
        nc.gpsimd.affine_select(out=m[:, :], in_=m[:, :],
                                compare_op=ALU.is_ge, fill=0.0,
                                base=-d, channel_multiplier=-1,
                                pattern=[[1, 512]])
        masks.append(m)
    # combined masks for the augmented tile: rows 0:64 = d448 seq pattern,
    # rows 64:128 = depth (triangle for group 0, all-keep otherwise)
    maskA = const.tile([128, 512], BF16, tag="maskA", name="maskA")
    maskB = const.tile([128, 512], BF16, tag="maskB", name="maskB")
    nc.vector.tensor_copy(maskA[0:TD, :], masks[5][0:TD, :])
    nc.vector.tensor_copy(maskA[TD:128, :], masks[1][TD:128, :])
    nc.vector.tensor_copy(maskB[0:TD, :], masks[5][0:TD, :])
    nc.gpsimd.memset(maskB[TD:128, :], 1.0)

    # ------- weight lhsT tiles: pre-tiled bf16 inputs, contiguous loads ----
    def wtiles(src_4d, nrow_tiles, tag, eng):
        ts = []
        for m in range(nrow_tiles):
            row = []
            for kk in range(NKD):
                t = wpool.tile([128, 128], BF16, tag=f"{tag}{m}_{kk}",
                               name=f"{tag}{m}_{kk}")
                eng.dma_start(out=t[:, :], in_=src_4d[m, kk, :, :])
                row.append(t)
            ts.append(row)
        return ts

    wqT = wtiles(wq_t, HPC, "wq", nc.scalar)      # [2][16]
    wkT = wtiles(wk_t, 1, "wk", nc.scalar)[0]     # [16]
    wvT = wtiles(wv_t, 1, "wv", nc.scalar)[0]
    woT = None  # loaded lazily after the first attention group

    # ---------------- big persistent activations ----------------
    KT = [big.tile([HD, TD + T], BF16, tag=f"KT{b}", name=f"KT{b}")
          for b in range(B)]
    VC = [big.tile([128, 16 * 128], BF16, tag=f"VC{b}", name=f"VC{b}")
          for b in range(B)]
    VTs = [big.tile([HD, T], BF16, tag=f"VTs{b}", name=f"VTs{b}")
           for b in range(B)]
    # augmented last-diagonal tiles: [live 64 seq keys | 64 depth keys]
    KTa = [[big.tile([HD, 128], BF16, tag=f"KTa{b}_{g}", name=f"KTa{b}_{g}")
            for g in range(4)] for b in range(B)]
    VCa = [[big.tile([128, HD], BF16, tag=f"VCa{b}_{g}", name=f"VCa{b}_{g}")
            for g in range(4)] for b in range(B)]
    for b in range(B):
        nc.sync.dma_start(out=KT[b][:, 0:TD], in_=dkT_c[b, :, :])
        for g in range(4):
            nc.sync.dma_start(out=VCa[b][g][TD:128, :], in_=dv_c[b, :, :])

    Qcur = [None, None]  # per-chunk normalized Q tiles (h=0,1)
    xt_cur = [None]      # prefetched x^T stripes for the current chunk

    def load_stripes(c):
        ts = [xp.tile([128, 512], BF16, tag=f"xt{kk}", name=f"xt{kk}")
              for kk in range(NKD)]
        for kk in range(NKD):
            nc.sync.dma_start(out=ts[kk][:, :],
                              in_=x_bf[c, 128 * kk:128 * (kk + 1), :])
        return ts

    def rsqrt_scaled(ps_sq_src, out_ri, ln_bias):
        """out_ri = exp(-0.5*ln(sum_sq) + ln_bias) = scale/sqrt(sum_sq).
        ps_sq_src is a PSUM [128,512] with per-token sum of squares
        (all partitions identical)."""
        lnss = rp.tile([128, 512], F32, tag="lnss", name="lnss")
        nc.scalar.activation(lnss[:, :], ps_sq_src[:, :], AF.Ln,
                             bias=eps_sb[:, :])
        nc.scalar.activation(out_ri, lnss[:, :], AF.Exp, scale=-0.5,
                             bias=ln_bias)

    def rope(ps, n, out_tag):
        """psum [128,512] head-dim-major proj -> rope -> bf16 tile qr."""
        cs = cos_sb[:, 512 * n:512 * (n + 1)]
        sn = sin_sb[:, 512 * n:512 * (n + 1)]
        qb = rp.tile([128, 512], BF16, tag="qb", name="qb")
        nc.scalar.copy(qb[:, :], ps[:, :])
        swp = rp.tile([128, 512], BF16, tag="swp", name="swp")
        mask32 = []
        for j in range(16):
            mask32 += [2 * j + 1, 2 * j]
        nc.vector.stream_shuffle(swp[:, :], qb[:, :], mask32)
        m1 = rp.tile([128, 512], BF16, tag="m1", name="m1")
        nc.vector.tensor_mul(m1[:, :], ps[:, :], cs)
        m2 = rp.tile([128, 512], BF16, tag="m2", name="m2")
        nc.vector.tensor_mul(m2[:, :], swp[:, :], sn)
        qr = rp.tile([128, 512], BF16, tag=out_tag, name=out_tag)
        nc.vector.tensor_add(qr[:, :], m1[:, :], m2[:, :])
        return qr

    # =========================================================
    # per-chunk emission
    # =========================================================
    def proj_chunk(b, n):
        r0 = b * T + 512 * n
        xt = xt_cur[0]  # stripes prefetched by the previous chunk

        # emit the 4 projection chains; the sum-of-squares matmuls for
        # chain i are emitted during chain i+1 so PE never waits on the
        # scalar-engine Square.
        ps_q = []
        pend = []  # (ss_mm_args) queued one chain behind

        def flush_pend():
            while pend:
                q2t, ss = pend.pop(0)
                nc.tensor.matmul(ss[:, :], ones_bf[:, :], q2t[:, :],
                                 start=True, stop=True)

        qr_q, ri_q = [], []
        for h in range(HPC):
            ps = pp.tile([128, 512], F32, tag="psq", name="psq")
            for kk in range(NKD):
                nc.tensor.matmul(ps[:, :], wqT[h][kk][:, :], xt[kk][:, :],
                                 start=(kk == 0), stop=(kk == NKD - 1))
            flush_pend()
            qr = rope(ps, n, "qr")
            ss = pt.tile([128, 512], F32, tag="ss", name="ss")
            q2t = rp.tile([128, 512], BF16, tag="q2h", name="q2h")
            nc.scalar.activation(q2t[:, :], ps[:, :], AF.Square)
            pend.append((q2t, ss))
            qr_q.append(qr)
            ri = rp.tile([128, 512], F32, tag="ri", name="ri")
            ri_q.append((ss, ri))
            ps_q.append(ps)

        # k chain
        ps_k = pp.tile([128, 512], F32, tag="psq", name="psk")
        for kk in range(NKD):
            nc.tensor.matmul(ps_k[:, :], wkT[kk][:, :], xt[kk][:, :],
                             start=(kk == 0), stop=(kk == NKD - 1))
        flush_pend()
        qr_k = rope(ps_k, n, "qrk")
        ss_k = pt.tile([128, 512], F32, tag="ss", name="ssk")
        q2k = rp.tile([128, 512], BF16, tag="q2h", name="q2k")
        nc.scalar.activation(q2k[:, :], ps_k[:, :], AF.Square)
        pend.append((q2k, ss_k))

        # v chain
        ps_v = pp.tile([128, 512], F32, tag="psq", name="psv")
        for kk in range(NKD):
            nc.tensor.matmul(ps_v[:, :], wvT[kk][:, :], xt[kk][:, :],
                             start=(kk == 0), stop=(kk == NKD - 1))
        flush_pend()

        # q normalization: Q = qr * (qs/|q|), consumed by attn group g=n
        for h in range(HPC):
            ss, ri = ri_q[h]
            rsqrt_scaled(ss, ri[:, :], ln_qs[:, h:h + 1])
            Qcur[h] = qp.tile([HD, 512], BF16, tag=f"Q{h}", name=f"Q{h}")
            nc.vector.tensor_mul(Qcur[h][:, :], qr_q[h][:, :], ri[:, :])
        # k normalization: KT = qr_k * (ks/|k|)
        ri_k = rp.tile([128, 512], F32, tag="rik", name="rik")
        rsqrt_scaled(ss_k, ri_k[:, :], ln_ks[:, 0:1])
        nc.vector.tensor_mul(KT[b][:, TD + 512 * n:TD + 512 * (n + 1)],
                             qr_k[:, :], ri_k[:, :])

        # v: add bias, store v^T, roundtrip via DRAM for the transpose
        vbt_sb = vbp.tile([128, 512], F32, tag="vbts", name="vbt_sb")
        nc.sync.dma_start(out=vbt_sb[:, :], in_=vb_ch[NCH * b + n, :, :])
        nc.vector.tensor_add(VTs[b][:, 512 * n:512 * (n + 1)],
                             ps_v[:, :], vbt_sb[:, :])
        nc.gpsimd.dma_start(out=vt_dram[b, :, 512 * n:512 * (n + 1)],
                            in_=VTs[b][:, 512 * n:512 * (n + 1)])
        # V natural layout for this chunk's 4 key tiles (DMA transpose)
        for tt in range(4 * n, 4 * n + 4):
            nc.sync.dma_start(out=VC[b][:, 128 * tt:128 * (tt + 1)],
                              in_=vt_dram[b, :, 128 * tt:128 * (tt + 1)],
                              transpose=True)
        # augmented last-diagonal tiles for query group g=n
        g = n
        s0 = TD + 512 * g + 384
        nc.vector.tensor_copy(KTa[b][g][:, 0:TD], KT[b][:, s0:s0 + TD])
        nc.vector.tensor_copy(KTa[b][g][:, TD:128], KT[b][:, 0:TD])
        nc.vector.tensor_copy(
            VCa[b][g][0:TD, :],
            VC[b][0:TD, 128 * (4 * g + 3):128 * (4 * g + 4)])
        # prefetch next chunk's x^T stripes while attention for this one runs
        c_next = NCH * b + n + 1
        if c_next < B * NCH:
            xt_cur[0] = load_stripes(c_next)

    def attn_group(b, g):
        c = NCH * b + g
        nk = 4 * (g + 1)  # seq k-tiles of 128
        vTg = VTs[b][:, 512 * g:512 * (g + 1)]
        v2 = vt_sb.tile([128, 512], BF16, tag="v2", name="v2")
        nc.gpsimd.tensor_mul(v2[:, :], vTg, vTg)
        rv = vt_sb.tile([128, 512], F32, tag="rv", name="rv")

        for h in range(HPC):
            q_sl = Qcur[h][:, :]
            y_ps = py.tile([128, 512], F32, tag="y", name="y_ps")
            z_ps = pz.tile([128, 512], F32, tag="z", name="z_ps")
            # software-pipelined: L(kt) leads its z/y accumulation by one
            # tile so the scalar exp latency is hidden behind PE work.
            Ps = [None] * nk
            for kt in range(nk):
                last = kt == nk - 1
                kT_t = (KTa[b][g][:, :] if last else
                        KT[b][:, TD + 128 * kt:TD + 128 * (kt + 1)])
                L = pl.tile([128, 512], F32, tag="L", name="L")
                nc.tensor.matmul(L[:, :], kT_t, q_sl, start=True, stop=True)
                P = ap_sb.tile([128, 512], BF16, tag="P", bufs=4, name="P")
                nc.scalar.activation(P[:, :], L[:, :], AF.Exp, scale=SCALE)
                di = kt - 4 * g
                if last:
                    nc.vector.tensor_mul(
                        P[:, :], P[:, :], (maskA if g == 0 else maskB)[:, :])
                elif di >= -1:
                    nc.vector.tensor_mul(P[:, :], P[:, :],
                                         masks[di + 2][:, :])
                Ps[kt] = P
                if kt >= 1:
                    _accum(b, g, kt - 1, nk, Ps[kt - 1], y_ps, z_ps)
            _accum(b, g, nk - 1, nk, Ps[nk - 1], y_ps, z_ps)
            if h == 0:
                # |v|^2 norm matmul, deferred so PE never waits on the
                # v-bias add at group start
                vns = pt.tile([128, 512], F32, tag="ss", name="vns")
                nc.tensor.matmul(vns[:, :], ones_bf[:, :], v2[:, :],
                                 start=True, stop=True)
                nc.vector.reciprocal_approx_fast(out=rv[:, :], in_=vns[:, :])

            # softmax denom + XSA (from a copy, to free the PSUM banks fast)
            rz = ap_sb.tile([128, 512], F32, tag="rz", name="rz")
            nc.vector.reciprocal_approx_fast(out=rz[:, :], in_=z_ps[:, :])
            yc = ap_sb.tile([128, 512], F32, tag="yc", name="yc")
            nc.scalar.copy(yc[:, :], y_ps[:, :])
            yv = ap_sb.tile([128, 512], BF16, tag="yv", name="yv")
            nc.vector.tensor_mul(yv[:, :], yc[:, :], vTg)
            dot = pt.tile([128, 512], F32, tag="ss", name="dot")
            nc.tensor.matmul(dot[:, :], ones_bf[:, :], yv[:, :],
                             start=True, stop=True)
            coef = ap_sb.tile([128, 512], F32, tag="coef", name="coef")
            nc.vector.tensor_mul(coef[:, :], dot[:, :], rv[:, :])
            t1 = ap_sb.tile([128, 512], F32, tag="t1", name="t1")
            nc.gpsimd.tensor_mul(t1[:, :], coef[:, :], vTg)
            y1 = ap_sb.tile([128, 512], F32, tag="y1", name="y1")
            nc.vector.tensor_sub(y1[:, :], yc[:, :], t1[:, :])
            yf = ap_sb.tile([128, 512], BF16, tag="yf", name="yf")
            nc.gpsimd.tensor_mul(yf[:, :], y1[:, :], rz[:, :])
            nc.gpsimd.dma_start(
                out=y_mine[c // 2][128 * h:128 * (h + 1),
                                   (c % 2) * 512:(c % 2) * 512 + 512],
                in_=yf[:, :])

    def _accum(b, g, kt, nk, P, y_ps, z_ps):
        last = kt == nk - 1
        v_t = (VCa[b][g][:, :] if last else
               VC[b][:, 128 * kt:128 * kt + HD])
        nc.tensor.matmul(z_ps[:, :], ones_bf[:, :], P[:, :],
                         start=(kt == 0), stop=last)
        nc.tensor.matmul(y_ps[:, :], v_t, P[:, :],
                         start=(kt == 0), stop=last)

    def emit_ag(c):
        nc.gpsimd.collective_compute(
            "AllGather", ALU.bypass, replica_groups=[list(range(NCORES))],
            ins=[y_mine[c][:, :]], outs=[y_all[c][:, :]])

    def emit_wo(c):
        # one 512-token column chunk of the output; rhs from AG chunk c//2
        po = [pp.tile([128, 512], F32, tag="psq", name=f"po{m}")
              for m in range(HPC)]
        s0 = (c % 2) * 512
        for cc in range(NKD):
            t = wop.tile([128, 512], BF16, tag=f"yr{cc % 4}", name=f"yr{cc}")
            eng = nc.scalar if cc % 2 == 0 else nc.gpsimd
            eng.dma_start(out=t[:, :],
                          in_=y_all[c // 2][128 * cc:128 * (cc + 1),
                                            s0:s0 + 512])
            for m in range(HPC):
                nc.tensor.matmul(po[m][:, :], woT[m][cc][:, :], t[:, :],
                                 start=(cc == 0), stop=(cc == NKD - 1))
        for m in range(HPC):
            ob = wos.tile([128, 512], F32, tag="ob", name="ob")
            nc.vector.tensor_copy(ob[:, :], po[m][:, :])
            nc.sync.dma_start(
                out=outT[128 * m:128 * (m + 1), 512 * c:512 * (c + 1)],
                in_=ob[:, :])

    # =========================================================
    # pipeline
    # =========================================================
    xt_cur[0] = load_stripes(0)
    for b in range(B):
        for n in range(NCH):
            c = NCH * b + n
            proj_chunk(b, n)
            attn_group(b, n)
            if c % 2 == 1:
                emit_ag(c // 2)
            if c == 0:
                # late: keeps startup queues clear
                woT = wtiles(wo_t, HPC, "wo", nc.scalar)
            if c >= 2:
                emit_wo(c - 2)
    emit_wo(2 * NCH - 2)
    emit_wo(2 * NCH - 1)

    for p in (pz, py, pt, pl, pp, wos, wop, vt_sb, ap_sb, vbp, rp, qp, xp,
              big, wpool, const):
        p.release()


_NC_CACHE = None


def _get_nc():
    global _NC_CACHE
    if _NC_CACHE is None:
        _NC_CACHE = _build()
    return _NC_CACHE


def _tile_w(wT):
    """[DIM, OUT] f32 -> [OUT/128, DIM/128, 128, 128] contiguous bf16."""
    import ml_dtypes
    d, o = wT.shape
    t = wT.reshape(NKD, 128, o // 128, 128).transpose(2, 0, 1, 3)
    return np.ascontiguousarray(t.astype(ml_dtypes.bfloat16))


def _shard_inputs(inputs):
    import ml_dtypes
    BF = ml_dtypes.bfloat16
    x = np.asarray(inputs["x"], np.float32)
    fc = np.asarray(inputs["freqs_cos"], np.float32)
    fs = np.asarray(inputs["freqs_sin"], np.float32)
    vb = np.asarray(inputs["value_bias"], np.float32)
    dk = np.asarray(inputs["depth_k"], np.float32)
    dv = np.asarray(inputs["depth_v"], np.float32)
    wq = np.asarray(inputs["wq"], np.float32)
    wk = np.asarray(inputs["wk"], np.float32)
    wv = np.asarray(inputs["wv"], np.float32)
    wo = np.asarray(inputs["wo"], np.float32)
    qs = np.asarray(inputs["q_scale"], np.float32).reshape(H)
    ks = np.asarray(inputs["k_scale"], np.float32).reshape(KVH)

    xT = x.reshape(TOK, DIM).T                           # [DIM, TOK]
    x_bf = np.ascontiguousarray(
        xT.reshape(DIM, B * NCH, 512).transpose(1, 0, 2).astype(BF))
    cosT = np.ascontiguousarray(np.repeat(fc.T, 2, axis=0).astype(BF))
    sinT = np.repeat(fs.T, 2, axis=0).copy()
    sinT[0::2] *= -1.0
    sinT = np.ascontiguousarray(sinT.astype(BF))
    vbf = vb.reshape(TOK, KVH * HD)

    maps = []
    for c in range(NCORES):
        kvh = c // 2
        vbT = vbf[:, HD * kvh:HD * (kvh + 1)].T          # [HD, TOK]
        vb_ch = np.ascontiguousarray(
            vbT.reshape(HD, B * NCH, 512).transpose(1, 0, 2))
        m = {
            "x_bf": x_bf,
            "wq_t": _tile_w(wq[256 * c:256 * (c + 1)].T),
            "wk_t": _tile_w(wk[HD * kvh:HD * (kvh + 1)].T),
            "wv_t": _tile_w(wv[HD * kvh:HD * (kvh + 1)].T),
            "wo_t": _tile_w(wo[256 * c:256 * (c + 1)].T),
            "vb_ch": vb_ch,
            "dkT_c": np.ascontiguousarray(
                dk[:, kvh].transpose(0, 2, 1).astype(BF)),
            "dv_c": np.ascontiguousarray(dv[:, kvh].astype(BF)),
            "cosT": cosT,
            "sinT": sinT,
            "qs_c": np.ascontiguousarray(
                np.broadcast_to(qs[2 * c:2 * c + 2][None, :], (128, 2))),
            "ks_c": np.full((128, 1), ks[kvh], np.float32),
        }
        maps.append(m)
    return maps


def _gather_output(results):
    outT = np.concatenate([results[c]["outT"] for c in range(NCORES)], axis=0)
    return np.ascontiguousarray(outT.T).reshape(B, T, DIM).astype(np.float32)


def kernel(**inputs):
    from concourse import bass_utils
    nc = _get_nc()
    from concourse.bass_interp import get_hw_module
    maps = _shard_inputs(inputs)
    old = nc.m
    nc.m = get_hw_module(nc.m)
    try:
        res = bass_utils.run_bass_kernel_spmd(nc, maps, list(range(NCORES)))
    finally:
        nc.m = old
    return _gather_output(res.results)



# revision 18
# speedup vs baseline: 1.4677x; 1.4677x over previous
"""Trainium2 Bass kernel for nn_CodaAttention (GQA attention with depth-KV
prefix, QK-norm, RoPE, XSA value-projection subtraction).

Sharding: tensor-parallel over heads across 8 cores. Core c owns q-heads
{2c, 2c+1} and kv-head c//2. Pipeline over 512-token chunks:
projections(+RoPE/QK-norm) for chunk n, attention for query group n,
per-chunk AllGather of y, wo matmuls two chunks later.

v2 notes (vs v1):
- Weights / x / wo-rhs are host-packed so each SBUF load is one large
  contiguous DMA (6 weight DMAs instead of 96; 2 x-DMAs per chunk).
- Scalar engine runs ONLY Ln/Exp (one activation table set) - squares,
  copies and norm muls moved to DVE/GpSimd, killing ACT_TABLE_LOAD thrash.
- V transpose via PE (tensor.transpose) instead of a DRAM round-trip
  DMA-transpose.
- Attention uses the 64-shifted key-tile grid with EXACT causal masks:
  tile kt covers keys [128kt-64, 128kt+64) (tile 0 starts with the 64
  depth keys), diagonal tiles are column-narrowed, and the last 64 keys
  of each group go through a 64-partition augmented tile.
- Elementwise ops stay bf16-in-SBUF where possible (DVE 4x mode).
"""
import os
import sys

sys.path.insert(0, "/opt/trn_rl_repo")

import numpy as np

import concourse.bass as bass
import concourse.mybir as mybir
import concourse.tile as tile
from concourse import bacc

DT = mybir.dt
F32, BF16 = DT.float32, DT.bfloat16
AF = mybir.ActivationFunctionType
ALU = mybir.AluOpType

KDBG = int(os.environ.get("KDBG", "0"))

B, T, DIM = 2, 2048, 2048
H, KVH, HD = 16, 4, 128
TD = 64
NCORES = 8
HPC = H // NCORES            # q heads per core = 2
TOK = B * T                  # 4096 flattened tokens
NKD = DIM // 128             # 16 contraction tiles
NCH = T // 512               # 4 query groups (512-token chunks) per batch
SCALE = 1.0 / np.sqrt(HD)


def _build():
    nc = bacc.Bacc("TRN2", target_bir_lowering=False, debug=False,
                   num_devices=NCORES)

    def inp(name, shape, dt=F32):
        return nc.dram_tensor(name, list(shape), dt,
                              kind="ExternalInput").ap()

    # host-packed inputs (see _shard_inputs)
    x_bf = inp("x_bf", (B * NCH, 128, NKD * 512), BF16)
    wq_c = inp("wq_c", (HPC, 128, NKD * 128), BF16)
    wk_c = inp("wk_c", (128, NKD * 128), BF16)
    wv_c = inp("wv_c", (128, NKD * 128), BF16)
    wo_c = inp("wo_c", (HPC, 128, NKD * 128), BF16)
    vb_ch = inp("vb_ch", (B * NCH, HD, 512), BF16)   # value_bias^T chunks
    dkT_c = inp("dkT_c", (B, HD, TD), BF16)          # transposed depth_k
    dv_c = inp("dv_c", (B, TD, HD), BF16)
    cosT = inp("cosT", (HD, T), BF16)                # pair-duplicated cos
    sinT = inp("sinT", (HD, T), BF16)                # pair-dup sign-folded sin
    qs_c = inp("qs_c", (128, HPC))                   # q_scale per local head
    ks_c = inp("ks_c", (128, 1))                     # k_scale, bcast

    outT = nc.dram_tensor("outT", [HPC * HD, TOK], BF16,
                          kind="ExternalOutput").ap()
    dbg = nc.dram_tensor("dbg", [128, 8192], BF16,
                         kind="ExternalOutput").ap()

    # DRAM scratch: per-chunk y (AG input must be internal DRAM)
    y_mine = [nc.dram_tensor(f"y_mine{c}", [128, HPC, 512], BF16).ap()
              for c in range(B * NCH)]
    y_all = [nc.dram_tensor(f"y_all{c}", [NCORES, 128, HPC, 512], BF16,
                            addr_space="Shared").ap() for c in range(B * NCH)]

    with tile.TileContext(nc) as tc:
        _emit(nc, tc, locals())
    nc.compile()
    return nc


def _emit(nc, tc, v):
    x_bf, wq_c, wk_c, wv_c, wo_c = (v["x_bf"], v["wq_c"], v["wk_c"],
                                    v["wv_c"], v["wo_c"])
    vb_ch, dkT_c, dv_c, cosT, sinT = (v["vb_ch"], v["dkT_c"], v["dv_c"],
                                      v["cosT"], v["sinT"])
    qs_c, ks_c, outT = v["qs_c"], v["ks_c"], v["outT"]
    dbg = v["dbg"]
    y_mine, y_all = v["y_mine"], v["y_all"]

    # ---------------- pools ----------------
    const = tc.alloc_tile_pool(name="const", bufs=1)
    wpool = tc.alloc_tile_pool(name="wpool", bufs=1)
    big = tc.alloc_tile_pool(name="big", bufs=1)
    xp = tc.alloc_tile_pool(name="xp", bufs=2)
    rp = tc.alloc_tile_pool(name="rope", bufs=2)
    vbp = tc.alloc_tile_pool(name="vb", bufs=2)
    ap_sb = tc.alloc_tile_pool(name="attn_sb", bufs=2)
    wop = tc.alloc_tile_pool(name="wo_rhs", bufs=2)
    wos = tc.alloc_tile_pool(name="wo_sb", bufs=2)
    # PSUM: 8 banks total: pp 2 + pl 2 + pt 2 + py 1 + pz 1
    pp = tc.alloc_tile_pool(name="pp", bufs=2, space="PSUM")  # proj + wo
    pl = tc.alloc_tile_pool(name="pl", bufs=2, space="PSUM")  # logits + transp
    pt = tc.alloc_tile_pool(name="pt", bufs=2, space="PSUM")  # ss/vns/dot
    py = tc.alloc_tile_pool(name="py", bufs=1, space="PSUM")
    pz = tc.alloc_tile_pool(name="pz", bufs=1, space="PSUM")

    # ---------------- constants ----------------
    cos_sb = const.tile([HD, T], BF16, tag="cos")
    sin_sb = const.tile([HD, T], BF16, tag="sin")
    nc.sync.dma_start(out=cos_sb[:, :], in_=cosT[:, :])
    nc.sync.dma_start(out=sin_sb[:, :], in_=sinT[:, :])
    qs_sb = const.tile([128, HPC], F32, tag="qs")
    ks_sb = const.tile([128, 1], F32, tag="ks")
    nc.scalar.dma_start(out=qs_sb[:, :], in_=qs_c[:, :])
    nc.scalar.dma_start(out=ks_sb[:, :], in_=ks_c[:, :])
    ones_bf = const.tile([128, 128], BF16, tag="ones")
    nc.gpsimd.memset(ones_bf[:, :], 1.0)
    eps_sb = const.tile([128, 1], F32, tag="eps")
    nc.gpsimd.memset(eps_sb[:, :], 1e-12)
    ln_qs = const.tile([128, HPC], F32, tag="lnqs")
    nc.scalar.activation(ln_qs[:, :], qs_sb[:, :], AF.Ln)
    ln_ks = const.tile([128, 1], F32, tag="lnks")
    nc.scalar.activation(ln_ks[:, :], ks_sb[:, :], AF.Ln)

    # identity for PE transpose
    ident = const.tile([128, 128], BF16, tag="ident")
    nc.gpsimd.memset(ident[:, :], 1.0)
    nc.gpsimd.affine_select(out=ident[:, :], in_=ident[:, :],
                            compare_op=ALU.is_equal, fill=0.0,
                            base=0, channel_multiplier=-1,
                            pattern=[[1, 128]])

    # causal masks (keep where c >= p + d), 0/1 bf16
    def affmask(tag, d):
        m = const.tile([128, 512], BF16, tag=tag, name=tag)
        nc.gpsimd.memset(m[:, :], 1.0)
        nc.gpsimd.affine_select(out=m[:, :], in_=m[:, :],
                                compare_op=ALU.is_ge, fill=0.0,
                                base=-d, channel_multiplier=-1,
                                pattern=[[1, 512]])
        return m

    mask_m64 = affmask("m64", -64)   # di=0 tile: keep c >= p - 64
    mask_d0 = affmask("d0", 0)       # narrowed diag tiles: keep c' >= p

    # ------- weights: one contiguous DMA per [128, 2048] block ----------
    wq_sb = [wpool.tile([128, NKD * 128], BF16, tag=f"wq{m}", name=f"wq{m}")
             for m in range(HPC)]
    wk_sb = wpool.tile([128, NKD * 128], BF16, tag="wk")
    wv_sb = wpool.tile([128, NKD * 128], BF16, tag="wv")
    nc.scalar.dma_start(out=wq_sb[0][:, :], in_=wq_c[0, :, :])
    nc.sync.dma_start(out=wq_sb[1][:, :], in_=wq_c[1, :, :])
    nc.gpsimd.dma_start(out=wk_sb[:, :], in_=wk_c[:, :])
    nc.gpsimd.dma_start(out=wv_sb[:, :], in_=wv_c[:, :])
    wo_sb = None  # loaded after chunk 0

    # ---------------- big persistent activations ----------------
    # KT: col TD+s = seq key s (cols 0:TD = depth keys)
    KT = [big.tile([HD, TD + T], BF16, tag=f"KT{b}", name=f"KT{b}")
          for b in range(B)]
    # VC_sh: shifted V tiles; tile t rows = keys [128t-64, 128t+64);
    # tile 0 rows 0:64 = depth V.
    VC = [big.tile([128, 17 * 128], BF16, tag=f"VC{b}", name=f"VC{b}")
          for b in range(B)]
    VTs = [big.tile([HD, T], BF16, tag=f"VTs{b}", name=f"VTs{b}")
           for b in range(B)]
    for b in range(B):
        nc.sync.dma_start(out=KT[b][:, 0:TD], in_=dkT_c[b, :, :])
        nc.sync.dma_start(out=VC[b][0:TD, 0:128], in_=dv_c[b, :, :])

    Qcur = [None, None]
    xt_cur = [None]

    mask32 = []
    for j in range(16):
        mask32 += [2 * j + 1, 2 * j]

    def load_x(c):
        xt = xp.tile([128, NKD, 512], BF16, tag="xt", name="xt")
        nc.sync.dma_start(out=xt[:, 0:4, :], in_=x_bf[c, :, 0:4 * 512])
        nc.sync.dma_start(out=xt[:, 4:NKD, :], in_=x_bf[c, :, 4 * 512:])
        return xt

    def rsqrt_scaled(ss_ps, out_ri, ln_bias):
        """out_ri = exp(-0.5*ln(ss+eps) + ln_bias) = scale/sqrt(ss)."""
        lnss = rp.tile([128, 512], F32, tag="lnss", name="lnss")
        nc.scalar.activation(lnss[:, :], ss_ps[:, :], AF.Ln,
                             bias=eps_sb[:, :])
        nc.scalar.activation(out_ri, lnss[:, :], AF.Exp, scale=-0.5,
                             bias=ln_bias)

    def rope(qb, n, out_tag):
        """qb: bf16 SBUF copy of the projection (all ops SBUF/bf16)."""
        cs = cos_sb[:, 512 * n:512 * (n + 1)]
        sn = sin_sb[:, 512 * n:512 * (n + 1)]
        swp = rp.tile([128, 512], BF16, tag="swp", name="swp")
        nc.vector.stream_shuffle(swp[:, :], qb[:, :], mask32)
        m1 = rp.tile([128, 512], BF16, tag="m1", name="m1")
        nc.vector.tensor_mul(m1[:, :], qb[:, :], cs)
        m2 = rp.tile([128, 512], BF16, tag="m2", name="m2")
        nc.vector.tensor_mul(m2[:, :], swp[:, :], sn)
        qr = rp.tile([128, 512], BF16, tag=out_tag, name=out_tag)
        nc.vector.tensor_add(qr[:, :], m1[:, :], m2[:, :])
        return qr

    # =========================================================
    def proj_chunk(b, n):
        xt = xt_cur[0]
        pend = []       # (q2_tile, ss_psum) queued one chain behind
        normq = []      # ('q'|'k', h, qr, ss) normalized one chain behind

        def flush_pend():
            while pend:
                q2t, ss = pend.pop(0)
                nc.tensor.matmul(ss[:, :], ones_bf[:, :], q2t[:, :],
                                 start=True, stop=True)

        def norm_prev():
            # normalize the chain whose ss matmul was just flushed
            while normq:
                kind, h, qr, ss = normq.pop(0)
                if kind == "q":
                    ri = rp.tile([128, 512], BF16, tag="ri", name="ri")
                    rsqrt_scaled(ss, ri[:, :], ln_qs[:, h:h + 1])
                    Qcur[h] = rp.tile([HD, 512], BF16, tag=f"Q{h}",
                                      name=f"Q{h}")
                    nc.vector.tensor_mul(Qcur[h][:, :], qr[:, :], ri[:, :])
                else:
                    ri_k = rp.tile([128, 512], BF16, tag="rik", name="rik")
                    rsqrt_scaled(ss, ri_k[:, :], ln_ks[:, 0:1])
                    nc.vector.tensor_mul(
                        KT[b][:, TD + 512 * n:TD + 512 * (n + 1)],
                        qr[:, :], ri_k[:, :])

        def chain(w_ap, nm):
            ps = pp.tile([128, 512], F32, tag="psq", name=nm)
            for kk in range(NKD):
                nc.tensor.matmul(ps[:, :], w_ap[:, 128 * kk:128 * (kk + 1)],
                                 xt[:, kk, :],
                                 start=(kk == 0), stop=(kk == NKD - 1))
            flush_pend()
            return ps

        for h in range(HPC):
            ps = chain(wq_sb[h], "psq")
            qb = rp.tile([128, 512], BF16, tag="qb", name="qb")
            nc.vector.tensor_copy(qb[:, :], ps[:, :])
            qr = rope(qb, n, "qr")
            q2t = rp.tile([128, 512], BF16, tag="q2h", name="q2h")
            nc.vector.tensor_mul(q2t[:, :], qb[:, :], qb[:, :])
            ss = pt.tile([128, 512], F32, tag="ss", name="ss")
            pend.append((q2t, ss))
            norm_prev()
            normq.append(("q", h, qr, ss))

        ps_k = chain(wk_sb, "psk")
        kb = rp.tile([128, 512], BF16, tag="qb", name="kb")
        nc.vector.tensor_copy(kb[:, :], ps_k[:, :])
        qr_k = rope(kb, n, "qrk")
        q2k = rp.tile([128, 512], BF16, tag="q2h", name="q2k")
        nc.vector.tensor_mul(q2k[:, :], kb[:, :], kb[:, :])
        ss_k = pt.tile([128, 512], F32, tag="ss", name="ssk")
        pend.append((q2k, ss_k))
        norm_prev()
        normq.append(("k", 0, qr_k, ss_k))

        ps_v = chain(wv_sb, "psv")
        norm_prev()

        # v = proj + bias -> VTs (v^T), then PE-transpose into VC (shifted)
        vbt_sb = vbp.tile([128, 512], BF16, tag="vbts", name="vbt_sb")
        nc.sync.dma_start(out=vbt_sb[:, :], in_=vb_ch[NCH * b + n, :, :])
        nc.vector.tensor_add(VTs[b][:, 512 * n:512 * (n + 1)],
                             ps_v[:, :], vbt_sb[:, :])
        for j in range(4 * n, 4 * n + 4):
            tp = pl.tile([128, 128], BF16, tag="L", name="tp",
                         padded_shape=[128, 1024])
            nc.tensor.transpose(tp[:, 0:128],
                                VTs[b][:, 128 * j:128 * (j + 1)],
                                ident[:, :])
            # aligned token block j rows 0:64 -> shifted tile j rows 64:128
            nc.vector.tensor_copy(VC[b][64:128, 128 * j:128 * (j + 1)],
                                  tp[0:64, 0:128])
            # rows 64:128 -> shifted tile j+1 rows 0:64
            nc.vector.tensor_copy(VC[b][0:64, 128 * (j + 1):128 * (j + 2)],
                                  tp[64:128, 0:128])

        c_next = NCH * b + n + 1
        if c_next < B * NCH:
            xt_cur[0] = load_x(c_next)

    # =========================================================
    def attn_group(b, g):
        # Reference causal mask is top-left aligned on the CONCATENATED
        # [depth | seq] axis: query c attends concat position j <= c,
        # i.e. depth key j <= c and seq key s <= c - TD. On the shifted
        # tile grid (tile kt = concat positions [128kt, 128kt+128), with
        # partition p = concat pos 128kt + p) this is uniformly
        # "keep c >= p + 128*di": no mask below the diagonal, mask_d0 on
        # diagonal tiles, and narrowed slices above.
        c = NCH * b + g
        nseq = 4 * g + 4           # shifted seq tiles
        vTg = VTs[b][:, 512 * g:512 * (g + 1)]
        v2g = ap_sb.tile([128, 512], BF16, tag="v2", name="v2")
        nc.gpsimd.tensor_mul(v2g[:, :], vTg, vTg)
        rv = ap_sb.tile([128, 512], F32, tag="rv", name="rv")

        # tile descriptors: (lhsT_k, lhsT_v, q0, width, mask, npart)
        tiles = []
        for kt in range(nseq):
            di = kt - 4 * g
            kslc = KT[b][:, 128 * kt:128 * (kt + 1)]
            vslc = VC[b][:, 128 * kt:128 * (kt + 1)]
            if di < 0:
                tiles.append((kslc, vslc, 0, 512, None, 128))
            else:
                q0 = 128 * di
                tiles.append((kslc, vslc, q0, 512 - q0, mask_d0, 128))
        ntile = len(tiles)

        for h in range(HPC):
            q_sl = Qcur[h]
            y_ps = py.tile([128, 512], F32, tag="y", name="y_ps")
            z_ps = pz.tile([128, 512], F32, tag="z", name="z_ps")
            Ps = [None] * ntile

            def accum(i):
                _, vt, q0, w, _, npart = tiles[i]
                P = Ps[i]
                first, last = i == 0, i == ntile - 1
                nc.tensor.matmul(z_ps[:, q0:512], ones_bf[0:npart, :],
                                 P, start=first, stop=last)
                nc.tensor.matmul(y_ps[:, q0:512], vt, P,
                                 start=first, stop=last)

            for i, (kt_sl, vt, q0, w, mk, npart) in enumerate(tiles):
                L = pl.tile([128, 512], F32, tag="L", name="L")
                nc.tensor.matmul(L[0:npart, 0:w], kt_sl,
                                 q_sl[:, q0:512], start=True, stop=True)
                P = ap_sb.tile([128, 512], BF16, tag="P", bufs=4, name="P")
                nc.scalar.activation(P[0:npart, 0:w], L[0:npart, 0:w],
                                     AF.Exp, scale=SCALE)
                if mk is not None:
                    nc.vector.tensor_mul(P[0:npart, 0:w], P[0:npart, 0:w],
                                         mk[0:npart, 0:w])
                Ps[i] = P[0:npart, 0:w]
                if i >= 1:
                    accum(i - 1)
            accum(ntile - 1)

            if h == 0:
                vns = pt.tile([128, 512], F32, tag="ss", name="vns")
                nc.tensor.matmul(vns[:, :], ones_bf[:, :], v2g[:, :],
                                 start=True, stop=True)
                nc.vector.reciprocal_approx_fast(out=rv[:, :], in_=vns[:, :])

            rz = ap_sb.tile([128, 512], F32, tag="rz", name="rz")
            nc.vector.reciprocal_approx_fast(out=rz[:, :], in_=z_ps[:, :])
            yn = ap_sb.tile([128, 512], BF16, tag="yn", name="yn")
            nc.vector.tensor_mul(yn[:, :], y_ps[:, :], rz[:, :])
            yv = ap_sb.tile([128, 512], BF16, tag="yv", name="yv")
            nc.vector.tensor_mul(yv[:, :], yn[:, :], vTg)
            dot = pt.tile([128, 512], F32, tag="ss", name="dot")
            nc.tensor.matmul(dot[:, :], ones_bf[:, :], yv[:, :],
                             start=True, stop=True)
            coef = ap_sb.tile([128, 512], BF16, tag="coef", name="coef")
            nc.vector.tensor_mul(coef[:, :], dot[:, :], rv[:, :])
            t1 = ap_sb.tile([128, 512], BF16, tag="t1", name="t1")
            nc.vector.tensor_mul(t1[:, :], coef[:, :], vTg)
            yf = ap_sb.tile([128, 512], BF16, tag="yf", name="yf")
            nc.vector.tensor_sub(yf[:, :], yn[:, :], t1[:, :])
            nc.gpsimd.dma_start(out=y_mine[c][:, h, :], in_=yf[:, :])
            if KDBG and b == 0 and h == 0:
                nc.sync.dma_start(out=dbg[:, 512 * g:512 * (g + 1)],
                                  in_=yf[:, :])

    def emit_ag(c):
        nc.gpsimd.collective_compute(
            "AllGather", ALU.bypass, replica_groups=[list(range(NCORES))],
            ins=[y_mine[c][:, :, :]], outs=[y_all[c][:, :, :, :]])

    def emit_wo(c):
        yr = wop.tile([128, NCORES, HPC, 512], BF16, tag="yr", name="yr")
        for r in range(NCORES):
            eng = (nc.scalar, nc.sync, nc.gpsimd)[r % 3]
            eng.dma_start(out=yr[:, r, :, :], in_=y_all[c][r, :, :, :])
        po = [pp.tile([128, 512], F32, tag="psq", name=f"po{m}")
              for m in range(HPC)]
        for cc in range(NKD):
            rhs = yr[:, cc // 2, cc % 2, :]
            for m in range(HPC):
                nc.tensor.matmul(po[m][:, :],
                                 wo_sb[m][:, 128 * cc:128 * (cc + 1)],
                                 rhs, start=(cc == 0), stop=(cc == NKD - 1))
        if KDBG and c == 0:
            nc.sync.dma_start(out=dbg[:, 2048:2560], in_=yr[:, 1, 0, :])
            nc.sync.dma_start(out=dbg[:, 2560:3072], in_=yr[:, 3, 0, :])
            nc.sync.dma_start(out=dbg[:, 3072:3584], in_=yr[:, 6, 1, :])
        if KDBG and c == 1:
            nc.sync.dma_start(out=dbg[:, 4608:5120], in_=yr[:, 0, 0, :])
        for m in range(HPC):
            ob = wos.tile([128, 512], BF16, tag="ob", name="ob")
            nc.vector.tensor_copy(ob[:, :], po[m][:, :])
            if KDBG and c == 0:
                nc.sync.dma_start(out=dbg[:, 3584 + 512 * m:4096 + 512 * m],
                                  in_=ob[:, :])
            nc.sync.dma_start(
                out=outT[128 * m:128 * (m + 1), 512 * c:512 * (c + 1)],
                in_=ob[:, :])

    # =========================================================
    xt_cur[0] = load_x(0)
    for b in range(B):
        for n in range(NCH):
            c = NCH * b + n
            proj_chunk(b, n)
            attn_group(b, n)
            emit_ag(c)
            if c == 0:
                wo_sb = [wpool.tile([128, NKD * 128], BF16, tag=f"wo{m}",
                                    name=f"wo{m}") for m in range(HPC)]
                nc.scalar.dma_start(out=wo_sb[0][:, :], in_=wo_c[0, :, :])
                nc.sync.dma_start(out=wo_sb[1][:, :], in_=wo_c[1, :, :])
            if c >= 2:
                emit_wo(c - 2)
    emit_wo(2 * NCH - 2)
    emit_wo(2 * NCH - 1)

    for p in (pz, py, pt, pl, pp, wos, wop, ap_sb, vbp, rp, xp,
              big, wpool, const):
        p.release()


_NC_CACHE = None


def _get_nc():
    global _NC_CACHE
    if _NC_CACHE is None:
        _NC_CACHE = _build()
    return _NC_CACHE


def _pack_w(w_rows, nblk):
    """w_rows: [nblk*128 out-rows, DIM] f32 -> [nblk, 128, NKD*128] bf16
    packed so lhsT tile (m, kk) = buf[m][:, 128kk:128kk+128]."""
    import ml_dtypes
    s = w_rows.reshape(nblk, 128, NKD, 128)        # [m, col, kk, p]
    s = s.transpose(0, 3, 2, 1)                    # [m, p, kk, col]
    return np.ascontiguousarray(
        s.reshape(nblk, 128, NKD * 128).astype(ml_dtypes.bfloat16))


def _shard_inputs(inputs):
    import ml_dtypes
    BF = ml_dtypes.bfloat16
    x = np.asarray(inputs["x"], np.float32)
    fc = np.asarray(inputs["freqs_cos"], np.float32)
    fs = np.asarray(inputs["freqs_sin"], np.float32)
    vb = np.asarray(inputs["value_bias"], np.float32)
    dk = np.asarray(inputs["depth_k"], np.float32)
    dv = np.asarray(inputs["depth_v"], np.float32)
    wq = np.asarray(inputs["wq"], np.float32)
    wk = np.asarray(inputs["wk"], np.float32)
    wv = np.asarray(inputs["wv"], np.float32)
    wo = np.asarray(inputs["wo"], np.float32)
    qs = np.asarray(inputs["q_scale"], np.float32).reshape(H)
    ks = np.asarray(inputs["k_scale"], np.float32).reshape(KVH)

    xT = x.reshape(TOK, DIM).T                     # [DIM, TOK]
    # x_bf[c, p, kk*512+t] = xT[128kk+p, 512c+t]
    x_bf = np.ascontiguousarray(
        xT.reshape(NKD, 128, B * NCH, 512).transpose(2, 1, 0, 3)
        .reshape(B * NCH, 128, NKD * 512).astype(BF))
    cosT = np.ascontiguousarray(np.repeat(fc.T, 2, axis=0).astype(BF))
    sinT = np.repeat(fs.T, 2, axis=0).copy()
    sinT[0::2] *= -1.0
    sinT = np.ascontiguousarray(sinT.astype(BF))
    vbf = vb.reshape(TOK, KVH * HD)

    maps = []
    for c in range(NCORES):
        kvh = c // 2
        vbT = vbf[:, HD * kvh:HD * (kvh + 1)].T    # [HD, TOK]
        vb_c = np.ascontiguousarray(
            vbT.reshape(HD, B * NCH, 512).transpose(1, 0, 2).astype(BF))
        m = {
            "x_bf": x_bf,
            "wq_c": _pack_w(wq[256 * c:256 * (c + 1)], HPC),
            "wk_c": _pack_w(wk[HD * kvh:HD * (kvh + 1)], 1)[0],
            "wv_c": _pack_w(wv[HD * kvh:HD * (kvh + 1)], 1)[0],
            # wo: lhsT[p, col] = wo[256c+128m+col, 128cc+p] -> pack wo rows
            # like wq but with contraction = head-dim (wo columns)
            "wo_c": _pack_w(wo[256 * c:256 * (c + 1)], HPC),
            "vb_ch": vb_c,
            "dkT_c": np.ascontiguousarray(
                dk[:, kvh].transpose(0, 2, 1).astype(BF)),
            "dv_c": np.ascontiguousarray(dv[:, kvh].astype(BF)),
            "cosT": cosT,
            "sinT": sinT,
            "qs_c": np.ascontiguousarray(
                np.broadcast_to(qs[2 * c:2 * c + 2][None, :], (128, 2))),
            "ks_c": np.full((128, 1), ks[kvh], np.float32),
        }
        maps.append(m)
    return maps


def _gather_output(results):
    outT = np.concatenate(
        [np.asarray(results[c]["outT"], dtype=np.float32)
         for c in range(NCORES)], axis=0)
    return np.ascontiguousarray(outT.T).reshape(B, T, DIM).astype(np.float32)


def kernel(**inputs):
    from concourse import bass_utils
    nc = _get_nc()
    from concourse.bass_interp import get_hw_module
    maps = _shard_inputs(inputs)
    old = nc.m
    nc.m = get_hw_module(nc.m)
    try:
        res = bass_utils.run_bass_kernel_spmd(nc, maps, list(range(NCORES)))
    finally:
        nc.m = old
    return _gather_output(res.results)


# revision 22
# speedup vs baseline: 1.4779x; 1.0069x over previous
"""Trainium2 Bass kernel for nn_CodaAttention (GQA attention with depth-KV
prefix, QK-norm, RoPE, XSA value-projection subtraction).

Sharding: tensor-parallel over heads across 8 cores. Core c owns q-heads
{2c, 2c+1} and kv-head c//2. Pipeline over 512-token chunks:
projections(+RoPE/QK-norm) for chunk n, attention for query group n,
per-chunk AllGather of y, wo matmuls two chunks later.

v2 notes (vs v1):
- Weights / x / wo-rhs are host-packed so each SBUF load is one large
  contiguous DMA (6 weight DMAs instead of 96; 2 x-DMAs per chunk).
- Scalar engine runs ONLY Ln/Exp (one activation table set) - squares,
  copies and norm muls moved to DVE/GpSimd, killing ACT_TABLE_LOAD thrash.
- V transpose via PE (tensor.transpose) instead of a DRAM round-trip
  DMA-transpose.
- Attention uses the 64-shifted key-tile grid with EXACT causal masks:
  tile kt covers keys [128kt-64, 128kt+64) (tile 0 starts with the 64
  depth keys), diagonal tiles are column-narrowed, and the last 64 keys
  of each group go through a 64-partition augmented tile.
- Elementwise ops stay bf16-in-SBUF where possible (DVE 4x mode).
"""
import os
import sys

sys.path.insert(0, "/opt/trn_rl_repo")

import numpy as np

import concourse.bass as bass
import concourse.mybir as mybir
import concourse.tile as tile
from concourse import bacc

DT = mybir.dt
F32, BF16 = DT.float32, DT.bfloat16
AF = mybir.ActivationFunctionType
ALU = mybir.AluOpType

KDBG = int(os.environ.get("KDBG", "0"))

B, T, DIM = 2, 2048, 2048
H, KVH, HD = 16, 4, 128
TD = 64
NCORES = 8
HPC = H // NCORES            # q heads per core = 2
TOK = B * T                  # 4096 flattened tokens
NKD = DIM // 128             # 16 contraction tiles
NCH = T // 512               # 4 query groups (512-token chunks) per batch
SCALE = 1.0 / np.sqrt(HD)


def _build():
    nc = bacc.Bacc("TRN2", target_bir_lowering=False, debug=False,
                   num_devices=NCORES)

    def inp(name, shape, dt=F32):
        return nc.dram_tensor(name, list(shape), dt,
                              kind="ExternalInput").ap()

    # host-packed inputs (see _shard_inputs)
    x_bf = inp("x_bf", (B * NCH, 128, NKD * 512), BF16)
    wq_c = inp("wq_c", (HPC, 128, NKD * 128), BF16)
    wk_c = inp("wk_c", (128, NKD * 128), BF16)
    wv_c = inp("wv_c", (128, NKD * 128), BF16)
    wo_c = inp("wo_c", (HPC, 128, NKD * 128), BF16)
    vb_ch = inp("vb_ch", (B * NCH, HD, 512), BF16)   # value_bias^T chunks
    dkT_c = inp("dkT_c", (B, HD, TD), BF16)          # transposed depth_k
    dv_c = inp("dv_c", (B, TD, HD), BF16)
    cosT = inp("cosT", (HD, T), BF16)                # pair-duplicated cos
    sinT = inp("sinT", (HD, T), BF16)                # pair-dup sign-folded sin
    qs_c = inp("qs_c", (128, HPC))                   # q_scale per local head
    ks_c = inp("ks_c", (128, 1))                     # k_scale, bcast

    outT = nc.dram_tensor("outT", [HPC * HD, TOK], BF16,
                          kind="ExternalOutput").ap()
    dbg = nc.dram_tensor("dbg", [128, 8192], BF16,
                         kind="ExternalOutput").ap()

    # DRAM scratch: per-chunk y (AG input must be internal DRAM)
    y_mine = [nc.dram_tensor(f"y_mine{c}", [128, HPC, 512], BF16).ap()
              for c in range(B * NCH)]
    y_all = [nc.dram_tensor(f"y_all{c}", [NCORES, 128, HPC, 512], BF16,
                            addr_space="Shared").ap() for c in range(B * NCH)]

    with tile.TileContext(nc) as tc:
        _emit(nc, tc, locals())
    nc.compile()
    return nc


def _emit(nc, tc, v):
    x_bf, wq_c, wk_c, wv_c, wo_c = (v["x_bf"], v["wq_c"], v["wk_c"],
                                    v["wv_c"], v["wo_c"])
    vb_ch, dkT_c, dv_c, cosT, sinT = (v["vb_ch"], v["dkT_c"], v["dv_c"],
                                      v["cosT"], v["sinT"])
    qs_c, ks_c, outT = v["qs_c"], v["ks_c"], v["outT"]
    dbg = v["dbg"]
    y_mine, y_all = v["y_mine"], v["y_all"]

    # ---------------- pools ----------------
    const = tc.alloc_tile_pool(name="const", bufs=1)
    wpool = tc.alloc_tile_pool(name="wpool", bufs=1)
    big = tc.alloc_tile_pool(name="big", bufs=1)
    xp = tc.alloc_tile_pool(name="xp", bufs=2)
    rp = tc.alloc_tile_pool(name="rope", bufs=2)
    vbp = tc.alloc_tile_pool(name="vb", bufs=2)
    ap_sb = tc.alloc_tile_pool(name="attn_sb", bufs=2)
    wop = tc.alloc_tile_pool(name="wo_rhs", bufs=2)
    wos = tc.alloc_tile_pool(name="wo_sb", bufs=2)
    # PSUM: 8 banks total: pp 2 + pl 2 + pt 2 + py 1 + pz 1
    pp = tc.alloc_tile_pool(name="pp", bufs=2, space="PSUM")  # proj + wo
    pl = tc.alloc_tile_pool(name="pl", bufs=2, space="PSUM")  # logits + transp
    pt = tc.alloc_tile_pool(name="pt", bufs=2, space="PSUM")  # ss/vns/dot
    py = tc.alloc_tile_pool(name="py", bufs=1, space="PSUM")
    pz = tc.alloc_tile_pool(name="pz", bufs=1, space="PSUM")

    # ---------------- constants ----------------
    cos_sb = const.tile([HD, T], BF16, tag="cos")
    sin_sb = const.tile([HD, T], BF16, tag="sin")
    nc.sync.dma_start(out=cos_sb[:, :], in_=cosT[:, :])
    nc.sync.dma_start(out=sin_sb[:, :], in_=sinT[:, :])
    qs_sb = const.tile([128, HPC], F32, tag="qs")
    ks_sb = const.tile([128, 1], F32, tag="ks")
    nc.scalar.dma_start(out=qs_sb[:, :], in_=qs_c[:, :])
    nc.scalar.dma_start(out=ks_sb[:, :], in_=ks_c[:, :])
    ones_bf = const.tile([128, 128], BF16, tag="ones")
    nc.gpsimd.memset(ones_bf[:, :], 1.0)
    eps_sb = const.tile([128, 1], F32, tag="eps")
    nc.gpsimd.memset(eps_sb[:, :], 1e-12)
    ln_qs = const.tile([128, HPC], F32, tag="lnqs")
    nc.scalar.activation(ln_qs[:, :], qs_sb[:, :], AF.Ln)
    ln_ks = const.tile([128, 1], F32, tag="lnks")
    nc.scalar.activation(ln_ks[:, :], ks_sb[:, :], AF.Ln)

    # identity for PE transpose
    ident = const.tile([128, 128], BF16, tag="ident")
    nc.gpsimd.memset(ident[:, :], 1.0)
    nc.gpsimd.affine_select(out=ident[:, :], in_=ident[:, :],
                            compare_op=ALU.is_equal, fill=0.0,
                            base=0, channel_multiplier=-1,
                            pattern=[[1, 128]])

    # causal masks (keep where c >= p + d), 0/1 bf16
    def affmask(tag, d):
        m = const.tile([128, 512], BF16, tag=tag, name=tag)
        nc.gpsimd.memset(m[:, :], 1.0)
        nc.gpsimd.affine_select(out=m[:, :], in_=m[:, :],
                                compare_op=ALU.is_ge, fill=0.0,
                                base=-d, channel_multiplier=-1,
                                pattern=[[1, 512]])
        return m

    mask_m64 = affmask("m64", -64)   # di=0 tile: keep c >= p - 64
    mask_d0 = affmask("d0", 0)       # narrowed diag tiles: keep c' >= p

    # ------- weights: one contiguous DMA per [128, 2048] block ----------
    wq_sb = [wpool.tile([128, NKD * 128], BF16, tag=f"wq{m}", name=f"wq{m}")
             for m in range(HPC)]
    wk_sb = wpool.tile([128, NKD * 128], BF16, tag="wk")
    wv_sb = wpool.tile([128, NKD * 128], BF16, tag="wv")
    nc.scalar.dma_start(out=wq_sb[0][:, :], in_=wq_c[0, :, :])
    nc.sync.dma_start(out=wq_sb[1][:, :], in_=wq_c[1, :, :])
    nc.gpsimd.dma_start(out=wk_sb[:, :], in_=wk_c[:, :])
    nc.gpsimd.dma_start(out=wv_sb[:, :], in_=wv_c[:, :])
    wo_sb = None  # loaded after chunk 0

    # ---------------- big persistent activations ----------------
    # KT: col TD+s = seq key s (cols 0:TD = depth keys)
    KT = [big.tile([HD, TD + T], BF16, tag=f"KT{b}", name=f"KT{b}")
          for b in range(B)]
    # VC_sh: shifted V tiles; tile t rows = keys [128t-64, 128t+64);
    # tile 0 rows 0:64 = depth V.
    VC = [big.tile([128, 17 * 128], BF16, tag=f"VC{b}", name=f"VC{b}")
          for b in range(B)]
    VTs = [big.tile([HD, T], BF16, tag=f"VTs{b}", name=f"VTs{b}")
           for b in range(B)]
    for b in range(B):
        nc.sync.dma_start(out=KT[b][:, 0:TD], in_=dkT_c[b, :, :])
        nc.sync.dma_start(out=VC[b][0:TD, 0:128], in_=dv_c[b, :, :])

    Qcur = [None, None]
    xt_cur = [None]

    mask32 = []
    for j in range(16):
        mask32 += [2 * j + 1, 2 * j]

    def load_x(c):
        xt = xp.tile([128, NKD, 512], BF16, tag="xt", name="xt")
        nc.gpsimd.dma_start(out=xt[:, 0:4, :], in_=x_bf[c, :, 0:4 * 512])
        nc.gpsimd.dma_start(out=xt[:, 4:NKD, :], in_=x_bf[c, :, 4 * 512:])
        return xt

    def rsqrt_scaled(ss_ps, out_ri, ln_bias):
        """out_ri = exp(-0.5*ln(ss+eps) + ln_bias) = scale/sqrt(ss)."""
        lnss = rp.tile([128, 512], F32, tag="lnss", name="lnss")
        nc.scalar.activation(lnss[:, :], ss_ps[:, :], AF.Ln,
                             bias=eps_sb[:, :])
        nc.scalar.activation(out_ri, lnss[:, :], AF.Exp, scale=-0.5,
                             bias=ln_bias)

    def rope(qb, n, out_tag):
        """qb: bf16 SBUF copy of the projection (all ops SBUF/bf16)."""
        cs = cos_sb[:, 512 * n:512 * (n + 1)]
        sn = sin_sb[:, 512 * n:512 * (n + 1)]
        swp = rp.tile([128, 512], BF16, tag="swp", name="swp")
        nc.vector.stream_shuffle(swp[:, :], qb[:, :], mask32)
        m1 = rp.tile([128, 512], BF16, tag="m1", name="m1")
        nc.vector.tensor_mul(m1[:, :], qb[:, :], cs)
        m2 = rp.tile([128, 512], BF16, tag="m2", name="m2")
        nc.vector.tensor_mul(m2[:, :], swp[:, :], sn)
        qr = rp.tile([128, 512], BF16, tag=out_tag, name=out_tag)
        nc.vector.tensor_add(qr[:, :], m1[:, :], m2[:, :])
        return qr

    # =========================================================
    def proj_chunk(b, n):
        xt = xt_cur[0]
        pend = []       # (q2_tile, ss_psum) queued one chain behind
        raw = []        # ('q'|'k', h, qr, ss): ss flushed, Ln pending
        lnq = []        # ('q'|'k', h, qr, lnss): Ln done, Exp pending

        def flush_pend():
            while pend:
                q2t, ss = pend.pop(0)
                nc.tensor.matmul(ss[:, :], ones_bf[:, :], q2t[:, :],
                                 start=True, stop=True)

        def ln_ready():
            # Ln for chains whose ss matmuls are flushed. All Lns of a
            # chunk are consecutive on the ACT queue (one table load);
            # the Exps run batched at end of chunk (one more).
            while raw:
                kind, h, qr, ss = raw.pop(0)
                lnss = rp.tile([128, 512], F32, tag="lnss", name="lnss",
                               bufs=3)
                nc.scalar.activation(lnss[:, :], ss[:, :], AF.Ln,
                                     bias=eps_sb[:, :])
                lnq.append((kind, h, qr, lnss))

        def chain(w_ap, nm):
            ps = pp.tile([128, 512], F32, tag="psq", name=nm)
            for kk in range(NKD):
                nc.tensor.matmul(ps[:, :], w_ap[:, 128 * kk:128 * (kk + 1)],
                                 xt[:, kk, :],
                                 start=(kk == 0), stop=(kk == NKD - 1))
            flush_pend()
            ln_ready()
            return ps

        for h in range(HPC):
            ps = chain(wq_sb[h], "psq")
            qb = rp.tile([128, 512], BF16, tag="qb", name="qb")
            nc.vector.tensor_copy(qb[:, :], ps[:, :])
            qr = rope(qb, n, "qr")
            q2t = rp.tile([128, 512], BF16, tag="q2h", name="q2h")
            nc.vector.tensor_mul(q2t[:, :], qb[:, :], qb[:, :])
            ss = pt.tile([128, 512], F32, tag="ss", name="ss")
            pend.append((q2t, ss))
            raw.append(("q", h, qr, ss))

        ps_k = chain(wk_sb, "psk")
        kb = rp.tile([128, 512], BF16, tag="qb", name="kb")
        nc.vector.tensor_copy(kb[:, :], ps_k[:, :])
        qr_k = rope(kb, n, "qrk")
        q2k = rp.tile([128, 512], BF16, tag="q2h", name="q2k")
        nc.vector.tensor_mul(q2k[:, :], kb[:, :], kb[:, :])
        ss_k = pt.tile([128, 512], F32, tag="ss", name="ssk")
        pend.append((q2k, ss_k))
        raw.append(("k", 0, qr_k, ss_k))

        ps_v = chain(wv_sb, "psv")
        ln_ready()

        # batched Exps + normalization muls
        for kind, h, qr, lnss in lnq:
            if kind == "q":
                ri = rp.tile([128, 512], BF16, tag="ri", name="ri")
                nc.scalar.activation(ri[:, :], lnss[:, :], AF.Exp,
                                     scale=-0.5, bias=ln_qs[:, h:h + 1])
                Qcur[h] = rp.tile([HD, 512], BF16, tag=f"Q{h}",
                                  name=f"Q{h}")
                nc.vector.tensor_mul(Qcur[h][:, :], qr[:, :], ri[:, :])
            else:
                ri_k = rp.tile([128, 512], BF16, tag="rik", name="rik")
                nc.scalar.activation(ri_k[:, :], lnss[:, :], AF.Exp,
                                     scale=-0.5, bias=ln_ks[:, 0:1])
                nc.vector.tensor_mul(
                    KT[b][:, TD + 512 * n:TD + 512 * (n + 1)],
                    qr[:, :], ri_k[:, :])
        lnq.clear()

        # v = proj + bias -> VTs (v^T), then PE-transpose into VC (shifted)
        vbt_sb = vbp.tile([128, 512], BF16, tag="vbts", name="vbt_sb")
        nc.gpsimd.dma_start(out=vbt_sb[:, :], in_=vb_ch[NCH * b + n, :, :])
        nc.vector.tensor_add(VTs[b][:, 512 * n:512 * (n + 1)],
                             ps_v[:, :], vbt_sb[:, :])
        for j in range(4 * n, 4 * n + 4):
            tp = pl.tile([128, 128], BF16, tag="L", name="tp",
                         padded_shape=[128, 1024])
            nc.tensor.transpose(tp[:, 0:128],
                                VTs[b][:, 128 * j:128 * (j + 1)],
                                ident[:, :])
            # aligned token block j rows 0:64 -> shifted tile j rows 64:128
            nc.vector.tensor_copy(VC[b][64:128, 128 * j:128 * (j + 1)],
                                  tp[0:64, 0:128])
            # rows 64:128 -> shifted tile j+1 rows 0:64
            nc.vector.tensor_copy(VC[b][0:64, 128 * (j + 1):128 * (j + 2)],
                                  tp[64:128, 0:128])

        c_next = NCH * b + n + 1
        if c_next < B * NCH:
            xt_cur[0] = load_x(c_next)

    # =========================================================
    def attn_group(b, g):
        # Reference causal mask is top-left aligned on the CONCATENATED
        # [depth | seq] axis: query c attends concat position j <= c,
        # i.e. depth key j <= c and seq key s <= c - TD. On the shifted
        # tile grid (tile kt = concat positions [128kt, 128kt+128), with
        # partition p = concat pos 128kt + p) this is uniformly
        # "keep c >= p + 128*di": no mask below the diagonal, mask_d0 on
        # diagonal tiles, and narrowed slices above.
        c = NCH * b + g
        nseq = 4 * g + 4           # shifted seq tiles
        vTg = VTs[b][:, 512 * g:512 * (g + 1)]
        v2g = ap_sb.tile([128, 512], BF16, tag="v2", name="v2")
        nc.gpsimd.tensor_mul(v2g[:, :], vTg, vTg)
        rv = ap_sb.tile([128, 512], F32, tag="rv", name="rv")

        # tile descriptors: (lhsT_k, lhsT_v, q0, width, mask, npart)
        tiles = []
        for kt in range(nseq):
            di = kt - 4 * g
            kslc = KT[b][:, 128 * kt:128 * (kt + 1)]
            vslc = VC[b][:, 128 * kt:128 * (kt + 1)]
            if di < 0:
                tiles.append((kslc, vslc, 0, 512, None, 128))
            else:
                q0 = 128 * di
                tiles.append((kslc, vslc, q0, 512 - q0, mask_d0, 128))
        ntile = len(tiles)

        for h in range(HPC):
            q_sl = Qcur[h]
            y_ps = py.tile([128, 512], F32, tag="y", name="y_ps")
            z_ps = pz.tile([128, 512], F32, tag="z", name="z_ps")
            Ps = [None] * ntile
            zst = [False]      # z accumulation started
            qsum = [None, 0]   # running quad P-sum (full tiles), count

            def z_emit(rhs_ap, npart, q0, last):
                nc.tensor.matmul(z_ps[:, q0:512], ones_bf[0:npart, :],
                                 rhs_ap, start=(not zst[0]), stop=last)
                zst[0] = True

            def accum_y(i):
                _, vt, q0, w, _, npart = tiles[i]
                nc.tensor.matmul(y_ps[:, q0:512], vt, Ps[i],
                                 start=(i == 0), stop=(i == ntile - 1))

            for i, (kt_sl, vt, q0, w, mk, npart) in enumerate(tiles):
                L = pl.tile([128, 512], F32, tag="L", name="L")
                nc.tensor.matmul(L[0:npart, 0:w], kt_sl,
                                 q_sl[:, q0:512], start=True, stop=True)
                P = ap_sb.tile([128, 512], BF16, tag="P", bufs=4, name="P")
                nc.scalar.activation(P[0:npart, 0:w], L[0:npart, 0:w],
                                     AF.Exp, scale=SCALE)
                if mk is not None:
                    nc.vector.tensor_mul(P[0:npart, 0:w], P[0:npart, 0:w],
                                         mk[0:npart, 0:w])
                Ps[i] = P[0:npart, 0:w]
                if mk is None:
                    # full tile: fold 4 P's into one z matmul via DVE adds
                    if qsum[1] % 4 == 0:
                        qsum[0] = Ps[i]
                    else:
                        t = ap_sb.tile([128, 512], BF16, tag="Pq", bufs=3,
                                       name="Pq")
                        nc.vector.tensor_add(t[:, :], qsum[0], Ps[i])
                        qsum[0] = t[:, :]
                    qsum[1] += 1
                    if qsum[1] % 4 == 0:
                        z_emit(qsum[0], 128, 0, False)
                else:
                    z_emit(Ps[i], npart, q0, i == ntile - 1)
                if i >= 1:
                    accum_y(i - 1)
            accum_y(ntile - 1)

            if h == 0:
                vns = pt.tile([128, 512], F32, tag="ss", name="vns")
                nc.tensor.matmul(vns[:, :], ones_bf[:, :], v2g[:, :],
                                 start=True, stop=True)
                nc.vector.reciprocal_approx_fast(out=rv[:, :], in_=vns[:, :])

            rz = ap_sb.tile([128, 512], F32, tag="rz", name="rz")
            nc.vector.reciprocal_approx_fast(out=rz[:, :], in_=z_ps[:, :])
            yn = ap_sb.tile([128, 512], BF16, tag="yn", name="yn")
            nc.vector.tensor_mul(yn[:, :], y_ps[:, :], rz[:, :])
            yv = ap_sb.tile([128, 512], BF16, tag="yv", name="yv")
            nc.vector.tensor_mul(yv[:, :], yn[:, :], vTg)
            dot = pt.tile([128, 512], F32, tag="ss", name="dot")
            nc.tensor.matmul(dot[:, :], ones_bf[:, :], yv[:, :],
                             start=True, stop=True)
            coef = ap_sb.tile([128, 512], BF16, tag="coef", name="coef")
            nc.vector.tensor_mul(coef[:, :], dot[:, :], rv[:, :])
            t1 = ap_sb.tile([128, 512], BF16, tag="t1", name="t1")
            nc.vector.tensor_mul(t1[:, :], coef[:, :], vTg)
            yf = ap_sb.tile([128, 512], BF16, tag="yf", name="yf")
            nc.vector.tensor_sub(yf[:, :], yn[:, :], t1[:, :])
            nc.gpsimd.dma_start(out=y_mine[c][:, h, :], in_=yf[:, :])
            if KDBG and b == 0 and h == 0:
                nc.sync.dma_start(out=dbg[:, 512 * g:512 * (g + 1)],
                                  in_=yf[:, :])

    def emit_ag(c):
        nc.gpsimd.collective_compute(
            "AllGather", ALU.bypass, replica_groups=[list(range(NCORES))],
            ins=[y_mine[c][:, :, :]], outs=[y_all[c][:, :, :, :]])

    def load_rhs(c):
        yr = wop.tile([128, NCORES, HPC, 512], BF16, tag="yr", name="yr")
        for r in range(NCORES):
            nc.sync.dma_start(out=yr[:, r, :, :], in_=y_all[c][r, :, :, :])
        return yr

    def emit_wo(c, yr):
        po = [pp.tile([128, 512], F32, tag="psq", name=f"po{m}")
              for m in range(HPC)]
        for cc in range(NKD):
            rhs = yr[:, cc // 2, cc % 2, :]
            for m in range(HPC):
                nc.tensor.matmul(po[m][:, :],
                                 wo_sb[m][:, 128 * cc:128 * (cc + 1)],
                                 rhs, start=(cc == 0), stop=(cc == NKD - 1))
        if KDBG and c == 0:
            nc.sync.dma_start(out=dbg[:, 2048:2560], in_=yr[:, 1, 0, :])
            nc.sync.dma_start(out=dbg[:, 2560:3072], in_=yr[:, 3, 0, :])
            nc.sync.dma_start(out=dbg[:, 3072:3584], in_=yr[:, 6, 1, :])
        if KDBG and c == 1:
            nc.sync.dma_start(out=dbg[:, 4608:5120], in_=yr[:, 0, 0, :])
        for m in range(HPC):
            ob = wos.tile([128, 512], BF16, tag="ob", name="ob")
            nc.vector.tensor_copy(ob[:, :], po[m][:, :])
            if KDBG and c == 0:
                nc.gpsimd.dma_start(out=dbg[:, 3584 + 512 * m:4096 + 512 * m],
                                    in_=ob[:, :])
            nc.gpsimd.dma_start(
                out=outT[128 * m:128 * (m + 1), 512 * c:512 * (c + 1)],
                in_=ob[:, :])

    # =========================================================
    xt_cur[0] = load_x(0)
    rhs_t = {}
    for b in range(B):
        for n in range(NCH):
            c = NCH * b + n
            proj_chunk(b, n)
            attn_group(b, n)
            emit_ag(c)
            if c == 0:
                wo_sb = [wpool.tile([128, NKD * 128], BF16, tag=f"wo{m}",
                                    name=f"wo{m}") for m in range(HPC)]
                nc.scalar.dma_start(out=wo_sb[0][:, :], in_=wo_c[0, :, :])
                nc.sync.dma_start(out=wo_sb[1][:, :], in_=wo_c[1, :, :])
            if c >= 1:
                rhs_t[c - 1] = load_rhs(c - 1)
            if c >= 2:
                emit_wo(c - 2, rhs_t.pop(c - 2))
    rhs_t[7] = load_rhs(7)
    emit_wo(6, rhs_t.pop(6))
    emit_wo(7, rhs_t.pop(7))

    for p in (pz, py, pt, pl, pp, wos, wop, ap_sb, vbp, rp, xp,
              big, wpool, const):
        p.release()


_NC_CACHE = None


def _get_nc():
    global _NC_CACHE
    if _NC_CACHE is None:
        _NC_CACHE = _build()
    return _NC_CACHE


def _pack_w(w_rows, nblk):
    """w_rows: [nblk*128 out-rows, DIM] f32 -> [nblk, 128, NKD*128] bf16
    packed so lhsT tile (m, kk) = buf[m][:, 128kk:128kk+128]."""
    import ml_dtypes
    s = w_rows.reshape(nblk, 128, NKD, 128)        # [m, col, kk, p]
    s = s.transpose(0, 3, 2, 1)                    # [m, p, kk, col]
    return np.ascontiguousarray(
        s.reshape(nblk, 128, NKD * 128).astype(ml_dtypes.bfloat16))


def _shard_inputs(inputs):
    import ml_dtypes
    BF = ml_dtypes.bfloat16
    x = np.asarray(inputs["x"], np.float32)
    fc = np.asarray(inputs["freqs_cos"], np.float32)
    fs = np.asarray(inputs["freqs_sin"], np.float32)
    vb = np.asarray(inputs["value_bias"], np.float32)
    dk = np.asarray(inputs["depth_k"], np.float32)
    dv = np.asarray(inputs["depth_v"], np.float32)
    wq = np.asarray(inputs["wq"], np.float32)
    wk = np.asarray(inputs["wk"], np.float32)
    wv = np.asarray(inputs["wv"], np.float32)
    wo = np.asarray(inputs["wo"], np.float32)
    qs = np.asarray(inputs["q_scale"], np.float32).reshape(H)
    ks = np.asarray(inputs["k_scale"], np.float32).reshape(KVH)

    xT = x.reshape(TOK, DIM).T                     # [DIM, TOK]
    # x_bf[c, p, kk*512+t] = xT[128kk+p, 512c+t]
    x_bf = np.ascontiguousarray(
        xT.reshape(NKD, 128, B * NCH, 512).transpose(2, 1, 0, 3)
        .reshape(B * NCH, 128, NKD * 512).astype(BF))
    cosT = np.ascontiguousarray(np.repeat(fc.T, 2, axis=0).astype(BF))
    sinT = np.repeat(fs.T, 2, axis=0).copy()
    sinT[0::2] *= -1.0
    sinT = np.ascontiguousarray(sinT.astype(BF))
    vbf = vb.reshape(TOK, KVH * HD)

    maps = []
    for c in range(NCORES):
        kvh = c // 2
        vbT = vbf[:, HD * kvh:HD * (kvh + 1)].T    # [HD, TOK]
        vb_c = np.ascontiguousarray(
            vbT.reshape(HD, B * NCH, 512).transpose(1, 0, 2).astype(BF))
        m = {
            "x_bf": x_bf,
            "wq_c": _pack_w(wq[256 * c:256 * (c + 1)], HPC),
            "wk_c": _pack_w(wk[HD * kvh:HD * (kvh + 1)], 1)[0],
            "wv_c": _pack_w(wv[HD * kvh:HD * (kvh + 1)], 1)[0],
            # wo: lhsT[p, col] = wo[256c+128m+col, 128cc+p] -> pack wo rows
            # like wq but with contraction = head-dim (wo columns)
            "wo_c": _pack_w(wo[256 * c:256 * (c + 1)], HPC),
            "vb_ch": vb_c,
            "dkT_c": np.ascontiguousarray(
                dk[:, kvh].transpose(0, 2, 1).astype(BF)),
            "dv_c": np.ascontiguousarray(dv[:, kvh].astype(BF)),
            "cosT": cosT,
            "sinT": sinT,
            "qs_c": np.ascontiguousarray(
                np.broadcast_to(qs[2 * c:2 * c + 2][None, :], (128, 2))),
            "ks_c": np.full((128, 1), ks[kvh], np.float32),
        }
        maps.append(m)
    return maps


def _gather_output(results):
    outT = np.concatenate(
        [np.asarray(results[c]["outT"], dtype=np.float32)
         for c in range(NCORES)], axis=0)
    return np.ascontiguousarray(outT.T).reshape(B, T, DIM).astype(np.float32)


def kernel(**inputs):
    from concourse import bass_utils
    nc = _get_nc()
    from concourse.bass_interp import get_hw_module
    maps = _shard_inputs(inputs)
    old = nc.m
    nc.m = get_hw_module(nc.m)
    try:
        res = bass_utils.run_bass_kernel_spmd(nc, maps, list(range(NCORES)))
    finally:
        nc.m = old
    return _gather_output(res.results)


# revision 23
# speedup vs baseline: 1.5310x; 1.0359x over previous
"""Trainium2 Bass kernel for nn_CodaAttention (GQA attention with depth-KV
prefix, QK-norm, RoPE, XSA value-projection subtraction).

Sharding: tensor-parallel over heads across 8 cores. Core c owns q-heads
{2c, 2c+1} and kv-head c//2. Pipeline over 512-token chunks:
projections(+RoPE/QK-norm) for chunk n, attention for query group n,
per-chunk AllGather of y, wo matmuls two chunks later.

v2 notes (vs v1):
- Weights / x / wo-rhs are host-packed so each SBUF load is one large
  contiguous DMA (6 weight DMAs instead of 96; 2 x-DMAs per chunk).
- Scalar engine runs ONLY Ln/Exp (one activation table set) - squares,
  copies and norm muls moved to DVE/GpSimd, killing ACT_TABLE_LOAD thrash.
- V transpose via PE (tensor.transpose) instead of a DRAM round-trip
  DMA-transpose.
- Attention uses the 64-shifted key-tile grid with EXACT causal masks:
  tile kt covers keys [128kt-64, 128kt+64) (tile 0 starts with the 64
  depth keys), diagonal tiles are column-narrowed, and the last 64 keys
  of each group go through a 64-partition augmented tile.
- Elementwise ops stay bf16-in-SBUF where possible (DVE 4x mode).
"""
import os
import sys

sys.path.insert(0, "/opt/trn_rl_repo")

import numpy as np

import concourse.bass as bass
import concourse.mybir as mybir
import concourse.tile as tile
from concourse import bacc

DT = mybir.dt
F32, BF16 = DT.float32, DT.bfloat16
AF = mybir.ActivationFunctionType
ALU = mybir.AluOpType

KDBG = int(os.environ.get("KDBG", "0"))

B, T, DIM = 2, 2048, 2048
H, KVH, HD = 16, 4, 128
TD = 64
NCORES = 8
HPC = H // NCORES            # q heads per core = 2
TOK = B * T                  # 4096 flattened tokens
NKD = DIM // 128             # 16 contraction tiles
NCH = T // 512               # 4 query groups (512-token chunks) per batch
SCALE = 1.0 / np.sqrt(HD)


def _build():
    nc = bacc.Bacc("TRN2", target_bir_lowering=False, debug=False,
                   num_devices=NCORES)

    def inp(name, shape, dt=F32):
        return nc.dram_tensor(name, list(shape), dt,
                              kind="ExternalInput").ap()

    # host-packed inputs (see _shard_inputs)
    x_bf = inp("x_bf", (B * NCH, 128, NKD * 512), BF16)
    wq_c = inp("wq_c", (HPC, 128, NKD * 128), BF16)
    wk_c = inp("wk_c", (128, NKD * 128), BF16)
    wv_c = inp("wv_c", (128, NKD * 128), BF16)
    wo_c = inp("wo_c", (HPC, 128, NKD * 128), BF16)
    vb_ch = inp("vb_ch", (B * NCH, HD, 512), BF16)   # value_bias^T chunks
    dkT_c = inp("dkT_c", (B, HD, TD), BF16)          # transposed depth_k
    dv_c = inp("dv_c", (B, TD, HD), BF16)
    cosT = inp("cosT", (HD, T), BF16)                # pair-duplicated cos
    sinT = inp("sinT", (HD, T), BF16)                # pair-dup sign-folded sin
    qs_c = inp("qs_c", (128, HPC))                   # q_scale per local head
    ks_c = inp("ks_c", (128, 1))                     # k_scale, bcast

    outT = nc.dram_tensor("outT", [HPC * HD, TOK], BF16,
                          kind="ExternalOutput").ap()
    dbg = nc.dram_tensor("dbg", [128, 8192], BF16,
                         kind="ExternalOutput").ap()

    dum_i = nc.dram_tensor("dum_i", [128], BF16).ap()
    dum_o = nc.dram_tensor("dum_o", [NCORES * 128], BF16,
                           addr_space="Shared").ap()

    # DRAM scratch: per-chunk y (AG input must be internal DRAM)
    y_mine = [nc.dram_tensor(f"y_mine{c}", [128, HPC, 512], BF16).ap()
              for c in range(B * NCH)]
    y_all = [nc.dram_tensor(f"y_all{c}", [NCORES, 128, HPC, 512], BF16,
                            addr_space="Shared").ap() for c in range(B * NCH)]

    with tile.TileContext(nc) as tc:
        _emit(nc, tc, locals())
    nc.compile()
    return nc


def _emit(nc, tc, v):
    x_bf, wq_c, wk_c, wv_c, wo_c = (v["x_bf"], v["wq_c"], v["wk_c"],
                                    v["wv_c"], v["wo_c"])
    vb_ch, dkT_c, dv_c, cosT, sinT = (v["vb_ch"], v["dkT_c"], v["dv_c"],
                                      v["cosT"], v["sinT"])
    qs_c, ks_c, outT = v["qs_c"], v["ks_c"], v["outT"]
    dbg = v["dbg"]
    y_mine, y_all = v["y_mine"], v["y_all"]
    dum_i, dum_o = v["dum_i"], v["dum_o"]

    # fire a tiny AllGather immediately: the CC core takes ~80us to
    # process its first collective; absorb that latency up front so
    # AG(0) completes promptly.
    nc.gpsimd.collective_compute(
        "AllGather", ALU.bypass, replica_groups=[list(range(NCORES))],
        ins=[dum_i[:]], outs=[dum_o[:]])

    # ---------------- pools ----------------
    const = tc.alloc_tile_pool(name="const", bufs=1)
    wpool = tc.alloc_tile_pool(name="wpool", bufs=1)
    big = tc.alloc_tile_pool(name="big", bufs=1)
    xp = tc.alloc_tile_pool(name="xp", bufs=2)
    rp = tc.alloc_tile_pool(name="rope", bufs=2)
    vbp = tc.alloc_tile_pool(name="vb", bufs=2)
    ap_sb = tc.alloc_tile_pool(name="attn_sb", bufs=2)
    wop = tc.alloc_tile_pool(name="wo_rhs", bufs=2)
    wos = tc.alloc_tile_pool(name="wo_sb", bufs=2)
    # PSUM: 8 banks total: pp 2 + pl 2 + pt 2 + py 1 + pz 1
    pp = tc.alloc_tile_pool(name="pp", bufs=2, space="PSUM")  # proj + wo
    pl = tc.alloc_tile_pool(name="pl", bufs=2, space="PSUM")  # logits + transp
    pt = tc.alloc_tile_pool(name="pt", bufs=2, space="PSUM")  # ss/vns/dot
    py = tc.alloc_tile_pool(name="py", bufs=1, space="PSUM")
    pz = tc.alloc_tile_pool(name="pz", bufs=1, space="PSUM")

    # ---------------- constants ----------------
    cos_sb = const.tile([HD, T], BF16, tag="cos")
    sin_sb = const.tile([HD, T], BF16, tag="sin")
    nc.sync.dma_start(out=cos_sb[:, :], in_=cosT[:, :])
    nc.sync.dma_start(out=sin_sb[:, :], in_=sinT[:, :])
    qs_sb = const.tile([128, HPC], F32, tag="qs")
    ks_sb = const.tile([128, 1], F32, tag="ks")
    nc.scalar.dma_start(out=qs_sb[:, :], in_=qs_c[:, :])
    nc.scalar.dma_start(out=ks_sb[:, :], in_=ks_c[:, :])
    ones_bf = const.tile([128, 128], BF16, tag="ones")
    nc.gpsimd.memset(ones_bf[:, :], 1.0)
    eps_sb = const.tile([128, 1], F32, tag="eps")
    nc.gpsimd.memset(eps_sb[:, :], 1e-12)
    # ones scaled by 1/qs^2 (per local q head) and 1/ks^2: folding the
    # norm scale into the sum-of-squares matmul makes the rsqrt
    # exp(-0.5*ln(ss/qs^2)) = qs/sqrt(ss) bias-free, so ONE wide Exp
    # covers all three chains (forces Ln/Ln/Ln/Exp table batching).
    sqq = const.tile([128, HPC], F32, tag="sqq")
    nc.vector.tensor_mul(sqq[:, :], qs_sb[:, :], qs_sb[:, :])
    rqq = const.tile([128, HPC], F32, tag="rqq")
    nc.vector.reciprocal(out=rqq[:, :], in_=sqq[:, :])
    sqk = const.tile([128, 1], F32, tag="sqk")
    nc.vector.tensor_mul(sqk[:, :], ks_sb[:, :], ks_sb[:, :])
    rqk = const.tile([128, 1], F32, tag="rqk")
    nc.vector.reciprocal(out=rqk[:, :], in_=sqk[:, :])
    ones_q = []
    for h in range(HPC):
        t = const.tile([128, 128], BF16, tag=f"onq{h}", name=f"onq{h}")
        nc.scalar.activation(t[:, :], ones_bf[:, :], AF.Copy,
                             scale=rqq[:, h:h + 1])
        ones_q.append(t)
    ones_k = const.tile([128, 128], BF16, tag="onk")
    nc.scalar.activation(ones_k[:, :], ones_bf[:, :], AF.Copy,
                         scale=rqk[:, 0:1])

    # identity for PE transpose
    ident = const.tile([128, 128], BF16, tag="ident")
    nc.gpsimd.memset(ident[:, :], 1.0)
    nc.gpsimd.affine_select(out=ident[:, :], in_=ident[:, :],
                            compare_op=ALU.is_equal, fill=0.0,
                            base=0, channel_multiplier=-1,
                            pattern=[[1, 128]])

    # causal masks (keep where c >= p + d), 0/1 bf16
    def affmask(tag, d):
        m = const.tile([128, 512], BF16, tag=tag, name=tag)
        nc.gpsimd.memset(m[:, :], 1.0)
        nc.gpsimd.affine_select(out=m[:, :], in_=m[:, :],
                                compare_op=ALU.is_ge, fill=0.0,
                                base=-d, channel_multiplier=-1,
                                pattern=[[1, 512]])
        return m

    mask_m64 = affmask("m64", -64)   # di=0 tile: keep c >= p - 64
    mask_d0 = affmask("d0", 0)       # narrowed diag tiles: keep c' >= p

    # ------- weights: one contiguous DMA per [128, 2048] block ----------
    wq_sb = [wpool.tile([128, NKD * 128], BF16, tag=f"wq{m}", name=f"wq{m}")
             for m in range(HPC)]
    wk_sb = wpool.tile([128, NKD * 128], BF16, tag="wk")
    wv_sb = wpool.tile([128, NKD * 128], BF16, tag="wv")
    nc.scalar.dma_start(out=wq_sb[0][:, :], in_=wq_c[0, :, :])
    nc.sync.dma_start(out=wq_sb[1][:, :], in_=wq_c[1, :, :])
    nc.gpsimd.dma_start(out=wk_sb[:, :], in_=wk_c[:, :])
    nc.gpsimd.dma_start(out=wv_sb[:, :], in_=wv_c[:, :])
    wo_sb = None  # loaded after chunk 0

    # ---------------- big persistent activations ----------------
    # KT: col TD+s = seq key s (cols 0:TD = depth keys)
    KT = [big.tile([HD, TD + T], BF16, tag=f"KT{b}", name=f"KT{b}")
          for b in range(B)]
    # VC_sh: shifted V tiles; tile t rows = keys [128t-64, 128t+64);
    # tile 0 rows 0:64 = depth V.
    VC = [big.tile([128, 17 * 128], BF16, tag=f"VC{b}", name=f"VC{b}")
          for b in range(B)]
    VTs = [big.tile([HD, T], BF16, tag=f"VTs{b}", name=f"VTs{b}")
           for b in range(B)]
    for b in range(B):
        nc.sync.dma_start(out=KT[b][:, 0:TD], in_=dkT_c[b, :, :])
        nc.sync.dma_start(out=VC[b][0:TD, 0:128], in_=dv_c[b, :, :])

    Qcur = [None, None]
    xt_cur = [None]

    mask32 = []
    for j in range(16):
        mask32 += [2 * j + 1, 2 * j]

    def load_x(c):
        xt = xp.tile([128, NKD, 512], BF16, tag="xt", name="xt")
        nc.gpsimd.dma_start(out=xt[:, 0:4, :], in_=x_bf[c, :, 0:4 * 512])
        nc.gpsimd.dma_start(out=xt[:, 4:NKD, :], in_=x_bf[c, :, 4 * 512:])
        return xt

    def rsqrt_scaled(ss_ps, out_ri, ln_bias):
        """out_ri = exp(-0.5*ln(ss+eps) + ln_bias) = scale/sqrt(ss)."""
        lnss = rp.tile([128, 512], F32, tag="lnss", name="lnss")
        nc.scalar.activation(lnss[:, :], ss_ps[:, :], AF.Ln,
                             bias=eps_sb[:, :])
        nc.scalar.activation(out_ri, lnss[:, :], AF.Exp, scale=-0.5,
                             bias=ln_bias)

    def rope(qb, n, out_tag):
        """qb: bf16 SBUF copy of the projection (all ops SBUF/bf16)."""
        cs = cos_sb[:, 512 * n:512 * (n + 1)]
        sn = sin_sb[:, 512 * n:512 * (n + 1)]
        swp = rp.tile([128, 512], BF16, tag="swp", name="swp")
        nc.vector.stream_shuffle(swp[:, :], qb[:, :], mask32)
        m1 = rp.tile([128, 512], BF16, tag="m1", name="m1")
        nc.vector.tensor_mul(m1[:, :], qb[:, :], cs)
        m2 = rp.tile([128, 512], BF16, tag="m2", name="m2")
        nc.vector.tensor_mul(m2[:, :], swp[:, :], sn)
        qr = rp.tile([128, 512], BF16, tag=out_tag, name=out_tag)
        nc.vector.tensor_add(qr[:, :], m1[:, :], m2[:, :])
        return qr

    # =========================================================
    def proj_chunk(b, n):
        xt = xt_cur[0]
        pend = []       # (q2_tile, ss_psum, ones_lhsT) queued one behind
        raw = []        # ('q'|'k', h, qr, ss): ss flushed, Ln pending
        lnq = []        # ('q'|'k', h, qr, slot): Ln done into lnc slot
        lnc = rp.tile([128, 3 * 512], F32, tag="lnc", name="lnc")

        def flush_pend():
            while pend:
                q2t, ss, oz = pend.pop(0)
                nc.tensor.matmul(ss[:, :], oz[:, :], q2t[:, :],
                                 start=True, stop=True)

        def ln_ready():
            # Ln for chains whose ss matmuls are flushed, written into
            # slots of ONE tile: the single wide Exp below reads the
            # whole tile, forcing Ln/Ln/Ln/Exp table batching.
            while raw:
                kind, h, qr, ss = raw.pop(0)
                slot = len(lnq)
                nc.scalar.activation(lnc[:, 512 * slot:512 * (slot + 1)],
                                     ss[:, :], AF.Ln, bias=eps_sb[:, :])
                lnq.append((kind, h, qr, slot))

        def chain(w_ap, nm):
            ps = pp.tile([128, 512], F32, tag="psq", name=nm)
            for kk in range(NKD):
                nc.tensor.matmul(ps[:, :], w_ap[:, 128 * kk:128 * (kk + 1)],
                                 xt[:, kk, :],
                                 start=(kk == 0), stop=(kk == NKD - 1))
            flush_pend()
            ln_ready()
            return ps

        for h in range(HPC):
            ps = chain(wq_sb[h], "psq")
            qb = rp.tile([128, 512], BF16, tag="qb", name="qb")
            nc.vector.tensor_copy(qb[:, :], ps[:, :])
            qr = rope(qb, n, "qr")
            q2t = rp.tile([128, 512], BF16, tag="q2h", name="q2h")
            nc.vector.tensor_mul(q2t[:, :], qb[:, :], qb[:, :])
            ss = pt.tile([128, 512], F32, tag="ss", name="ss")
            pend.append((q2t, ss, ones_q[h]))
            raw.append(("q", h, qr, ss))

        ps_k = chain(wk_sb, "psk")
        kb = rp.tile([128, 512], BF16, tag="qb", name="kb")
        nc.vector.tensor_copy(kb[:, :], ps_k[:, :])
        qr_k = rope(kb, n, "qrk")
        q2k = rp.tile([128, 512], BF16, tag="q2h", name="q2k")
        nc.vector.tensor_mul(q2k[:, :], kb[:, :], kb[:, :])
        ss_k = pt.tile([128, 512], F32, tag="ss", name="ssk")
        pend.append((q2k, ss_k, ones_k))
        raw.append(("k", 0, qr_k, ss_k))

        ps_v = chain(wv_sb, "psv")
        ln_ready()

        # ONE wide Exp for all three rsqrt chains, then the norm muls
        ri_all = rp.tile([128, 3 * 512], BF16, tag="riall", name="ri_all")
        nc.scalar.activation(ri_all[:, :], lnc[:, :], AF.Exp, scale=-0.5)
        for kind, h, qr, slot in lnq:
            ri = ri_all[:, 512 * slot:512 * (slot + 1)]
            if kind == "q":
                Qcur[h] = rp.tile([HD, 512], BF16, tag=f"Q{h}",
                                  name=f"Q{h}")
                nc.vector.tensor_mul(Qcur[h][:, :], qr[:, :], ri)
            else:
                nc.vector.tensor_mul(
                    KT[b][:, TD + 512 * n:TD + 512 * (n + 1)],
                    qr[:, :], ri)
        lnq.clear()

        # v = proj + bias -> VTs (v^T), then PE-transpose into VC (shifted)
        vbt_sb = vbp.tile([128, 512], BF16, tag="vbts", name="vbt_sb")
        nc.gpsimd.dma_start(out=vbt_sb[:, :], in_=vb_ch[NCH * b + n, :, :])
        nc.vector.tensor_add(VTs[b][:, 512 * n:512 * (n + 1)],
                             ps_v[:, :], vbt_sb[:, :])
        for j in range(4 * n, 4 * n + 4):
            tp = pl.tile([128, 128], BF16, tag="L", name="tp",
                         padded_shape=[128, 1024])
            nc.tensor.transpose(tp[:, 0:128],
                                VTs[b][:, 128 * j:128 * (j + 1)],
                                ident[:, :])
            # aligned token block j rows 0:64 -> shifted tile j rows 64:128
            nc.vector.tensor_copy(VC[b][64:128, 128 * j:128 * (j + 1)],
                                  tp[0:64, 0:128])
            # rows 64:128 -> shifted tile j+1 rows 0:64
            nc.vector.tensor_copy(VC[b][0:64, 128 * (j + 1):128 * (j + 2)],
                                  tp[64:128, 0:128])

        c_next = NCH * b + n + 1
        if c_next < B * NCH:
            xt_cur[0] = load_x(c_next)

    # =========================================================
    def attn_group(b, g):
        # Reference causal mask is top-left aligned on the CONCATENATED
        # [depth | seq] axis: query c attends concat position j <= c,
        # i.e. depth key j <= c and seq key s <= c - TD. On the shifted
        # tile grid (tile kt = concat positions [128kt, 128kt+128), with
        # partition p = concat pos 128kt + p) this is uniformly
        # "keep c >= p + 128*di": no mask below the diagonal, mask_d0 on
        # diagonal tiles, and narrowed slices above.
        c = NCH * b + g
        nseq = 4 * g + 4           # shifted seq tiles
        vTg = VTs[b][:, 512 * g:512 * (g + 1)]
        v2g = ap_sb.tile([128, 512], BF16, tag="v2", name="v2")
        nc.gpsimd.tensor_mul(v2g[:, :], vTg, vTg)
        rv = ap_sb.tile([128, 512], F32, tag="rv", name="rv")

        # tile descriptors: (lhsT_k, lhsT_v, q0, width, mask, npart)
        tiles = []
        for kt in range(nseq):
            di = kt - 4 * g
            kslc = KT[b][:, 128 * kt:128 * (kt + 1)]
            vslc = VC[b][:, 128 * kt:128 * (kt + 1)]
            if di < 0:
                tiles.append((kslc, vslc, 0, 512, None, 128))
            else:
                q0 = 128 * di
                tiles.append((kslc, vslc, q0, 512 - q0, mask_d0, 128))
        ntile = len(tiles)

        for h in range(HPC):
            q_sl = Qcur[h]
            y_ps = py.tile([128, 512], F32, tag="y", name="y_ps")
            z_ps = pz.tile([128, 512], F32, tag="z", name="z_ps")
            Ps = [None] * ntile
            zst = [False]      # z accumulation started
            qsum = [None, 0]   # running quad P-sum (full tiles), count

            def z_emit(rhs_ap, npart, q0, last):
                nc.tensor.matmul(z_ps[:, q0:512], ones_bf[0:npart, :],
                                 rhs_ap, start=(not zst[0]), stop=last)
                zst[0] = True

            def accum_y(i):
                _, vt, q0, w, _, npart = tiles[i]
                nc.tensor.matmul(y_ps[:, q0:512], vt, Ps[i],
                                 start=(i == 0), stop=(i == ntile - 1))

            for i, (kt_sl, vt, q0, w, mk, npart) in enumerate(tiles):
                L = pl.tile([128, 512], F32, tag="L", name="L")
                nc.tensor.matmul(L[0:npart, 0:w], kt_sl,
                                 q_sl[:, q0:512], start=True, stop=True)
                P = ap_sb.tile([128, 512], BF16, tag="P", bufs=4, name="P")
                nc.scalar.activation(P[0:npart, 0:w], L[0:npart, 0:w],
                                     AF.Exp, scale=SCALE)
                if mk is not None:
                    nc.vector.tensor_mul(P[0:npart, 0:w], P[0:npart, 0:w],
                                         mk[0:npart, 0:w])
                Ps[i] = P[0:npart, 0:w]
                if mk is None:
                    # full tile: fold 4 P's into one z matmul via DVE adds
                    if qsum[1] % 4 == 0:
                        qsum[0] = Ps[i]
                    else:
                        t = ap_sb.tile([128, 512], BF16, tag="Pq", bufs=3,
                                       name="Pq")
                        nc.vector.tensor_add(t[:, :], qsum[0], Ps[i])
                        qsum[0] = t[:, :]
                    qsum[1] += 1
                    if qsum[1] % 4 == 0:
                        z_emit(qsum[0], 128, 0, False)
                else:
                    z_emit(Ps[i], npart, q0, i == ntile - 1)
                if i >= 1:
                    accum_y(i - 1)
            accum_y(ntile - 1)

            if h == 0:
                vns = pt.tile([128, 512], F32, tag="ss", name="vns")
                nc.tensor.matmul(vns[:, :], ones_bf[:, :], v2g[:, :],
                                 start=True, stop=True)
                nc.vector.reciprocal_approx_fast(out=rv[:, :], in_=vns[:, :])

            rz = ap_sb.tile([128, 512], F32, tag="rz", name="rz")
            nc.vector.reciprocal_approx_fast(out=rz[:, :], in_=z_ps[:, :])
            yn = ap_sb.tile([128, 512], BF16, tag="yn", name="yn")
            nc.vector.tensor_mul(yn[:, :], y_ps[:, :], rz[:, :])
            yv = ap_sb.tile([128, 512], BF16, tag="yv", name="yv")
            nc.vector.tensor_mul(yv[:, :], yn[:, :], vTg)
            dot = pt.tile([128, 512], F32, tag="ss", name="dot")
            nc.tensor.matmul(dot[:, :], ones_bf[:, :], yv[:, :],
                             start=True, stop=True)
            coef = ap_sb.tile([128, 512], BF16, tag="coef", name="coef")
            nc.vector.tensor_mul(coef[:, :], dot[:, :], rv[:, :])
            t1 = ap_sb.tile([128, 512], BF16, tag="t1", name="t1")
            nc.vector.tensor_mul(t1[:, :], coef[:, :], vTg)
            yf = ap_sb.tile([128, 512], BF16, tag="yf", name="yf")
            nc.vector.tensor_sub(yf[:, :], yn[:, :], t1[:, :])
            nc.gpsimd.dma_start(out=y_mine[c][:, h, :], in_=yf[:, :])
            if KDBG and b == 0 and h == 0:
                nc.sync.dma_start(out=dbg[:, 512 * g:512 * (g + 1)],
                                  in_=yf[:, :])

    def emit_ag(c):
        nc.gpsimd.collective_compute(
            "AllGather", ALU.bypass, replica_groups=[list(range(NCORES))],
            ins=[y_mine[c][:, :, :]], outs=[y_all[c][:, :, :, :]])

    def load_rhs(c):
        yr = wop.tile([128, NCORES, HPC, 512], BF16, tag="yr", name="yr")
        for r in range(NCORES):
            nc.sync.dma_start(out=yr[:, r, :, :], in_=y_all[c][r, :, :, :])
        return yr

    def emit_wo(c, yr):
        po = [pp.tile([128, 512], F32, tag="psq", name=f"po{m}")
              for m in range(HPC)]
        for cc in range(NKD):
            rhs = yr[:, cc // 2, cc % 2, :]
            for m in range(HPC):
                nc.tensor.matmul(po[m][:, :],
                                 wo_sb[m][:, 128 * cc:128 * (cc + 1)],
                                 rhs, start=(cc == 0), stop=(cc == NKD - 1))
        if KDBG and c == 0:
            nc.sync.dma_start(out=dbg[:, 2048:2560], in_=yr[:, 1, 0, :])
            nc.sync.dma_start(out=dbg[:, 2560:3072], in_=yr[:, 3, 0, :])
            nc.sync.dma_start(out=dbg[:, 3072:3584], in_=yr[:, 6, 1, :])
        if KDBG and c == 1:
            nc.sync.dma_start(out=dbg[:, 4608:5120], in_=yr[:, 0, 0, :])
        for m in range(HPC):
            ob = wos.tile([128, 512], BF16, tag="ob", name="ob")
            nc.vector.tensor_copy(ob[:, :], po[m][:, :])
            if KDBG and c == 0:
                nc.gpsimd.dma_start(out=dbg[:, 3584 + 512 * m:4096 + 512 * m],
                                    in_=ob[:, :])
            nc.gpsimd.dma_start(
                out=outT[128 * m:128 * (m + 1), 512 * c:512 * (c + 1)],
                in_=ob[:, :])

    # =========================================================
    xt_cur[0] = load_x(0)
    rhs_t = {}
    for b in range(B):
        for n in range(NCH):
            c = NCH * b + n
            proj_chunk(b, n)
            attn_group(b, n)
            emit_ag(c)
            if c == 0:
                wo_sb = [wpool.tile([128, NKD * 128], BF16, tag=f"wo{m}",
                                    name=f"wo{m}") for m in range(HPC)]
                nc.scalar.dma_start(out=wo_sb[0][:, :], in_=wo_c[0, :, :])
                nc.sync.dma_start(out=wo_sb[1][:, :], in_=wo_c[1, :, :])
            if c >= 1:
                rhs_t[c - 1] = load_rhs(c - 1)
            if c >= 2:
                emit_wo(c - 2, rhs_t.pop(c - 2))
    rhs_t[7] = load_rhs(7)
    emit_wo(6, rhs_t.pop(6))
    emit_wo(7, rhs_t.pop(7))

    for p in (pz, py, pt, pl, pp, wos, wop, ap_sb, vbp, rp, xp,
              big, wpool, const):
        p.release()


_NC_CACHE = None


def _get_nc():
    global _NC_CACHE
    if _NC_CACHE is None:
        _NC_CACHE = _build()
    return _NC_CACHE


def _pack_w(w_rows, nblk):
    """w_rows: [nblk*128 out-rows, DIM] f32 -> [nblk, 128, NKD*128] bf16
    packed so lhsT tile (m, kk) = buf[m][:, 128kk:128kk+128]."""
    import ml_dtypes
    s = w_rows.reshape(nblk, 128, NKD, 128)        # [m, col, kk, p]
    s = s.transpose(0, 3, 2, 1)                    # [m, p, kk, col]
    return np.ascontiguousarray(
        s.reshape(nblk, 128, NKD * 128).astype(ml_dtypes.bfloat16))


def _shard_inputs(inputs):
    import ml_dtypes
    BF = ml_dtypes.bfloat16
    x = np.asarray(inputs["x"], np.float32)
    fc = np.asarray(inputs["freqs_cos"], np.float32)
    fs = np.asarray(inputs["freqs_sin"], np.float32)
    vb = np.asarray(inputs["value_bias"], np.float32)
    dk = np.asarray(inputs["depth_k"], np.float32)
    dv = np.asarray(inputs["depth_v"], np.float32)
    wq = np.asarray(inputs["wq"], np.float32)
    wk = np.asarray(inputs["wk"], np.float32)
    wv = np.asarray(inputs["wv"], np.float32)
    wo = np.asarray(inputs["wo"], np.float32)
    qs = np.asarray(inputs["q_scale"], np.float32).reshape(H)
    ks = np.asarray(inputs["k_scale"], np.float32).reshape(KVH)

    xT = x.reshape(TOK, DIM).T                     # [DIM, TOK]
    # x_bf[c, p, kk*512+t] = xT[128kk+p, 512c+t]
    x_bf = np.ascontiguousarray(
        xT.reshape(NKD, 128, B * NCH, 512).transpose(2, 1, 0, 3)
        .reshape(B * NCH, 128, NKD * 512).astype(BF))
    cosT = np.ascontiguousarray(np.repeat(fc.T, 2, axis=0).astype(BF))
    sinT = np.repeat(fs.T, 2, axis=0).copy()
    sinT[0::2] *= -1.0
    sinT = np.ascontiguousarray(sinT.astype(BF))
    vbf = vb.reshape(TOK, KVH * HD)

    maps = []
    for c in range(NCORES):
        kvh = c // 2
        vbT = vbf[:, HD * kvh:HD * (kvh + 1)].T    # [HD, TOK]
        vb_c = np.ascontiguousarray(
            vbT.reshape(HD, B * NCH, 512).transpose(1, 0, 2).astype(BF))
        m = {
            "x_bf": x_bf,
            "wq_c": _pack_w(wq[256 * c:256 * (c + 1)], HPC),
            "wk_c": _pack_w(wk[HD * kvh:HD * (kvh + 1)], 1)[0],
            "wv_c": _pack_w(wv[HD * kvh:HD * (kvh + 1)], 1)[0],
            # wo: lhsT[p, col] = wo[256c+128m+col, 128cc+p] -> pack wo rows
            # like wq but with contraction = head-dim (wo columns)
            "wo_c": _pack_w(wo[256 * c:256 * (c + 1)], HPC),
            "vb_ch": vb_c,
            "dkT_c": np.ascontiguousarray(
                dk[:, kvh].transpose(0, 2, 1).astype(BF)),
            "dv_c": np.ascontiguousarray(dv[:, kvh].astype(BF)),
            "cosT": cosT,
            "sinT": sinT,
            "qs_c": np.ascontiguousarray(
                np.broadcast_to(qs[2 * c:2 * c + 2][None, :], (128, 2))),
            "ks_c": np.full((128, 1), ks[kvh], np.float32),
        }
        maps.append(m)
    return maps


def _gather_output(results):
    outT = np.concatenate(
        [np.asarray(results[c]["outT"], dtype=np.float32)
         for c in range(NCORES)], axis=0)
    return np.ascontiguousarray(outT.T).reshape(B, T, DIM).astype(np.float32)


def kernel(**inputs):
    from concourse import bass_utils
    nc = _get_nc()
    from concourse.bass_interp import get_hw_module
    maps = _shard_inputs(inputs)
    old = nc.m
    nc.m = get_hw_module(nc.m)
    try:
        res = bass_utils.run_bass_kernel_spmd(nc, maps, list(range(NCORES)))
    finally:
        nc.m = old
    return _gather_output(res.results)
